# revision 7
# baseline (speedup 1.0000x reference)
"""Trainium2 Bass kernel for nn_BezierButtress (Bernstein-basis permutation chains).

Math (per permutation chain p, over depth d = 0..31):
    S_mean <- (S_mean @ Wm_d) * B(x_{perm[p,d]})        (K=17 wide state)
    S_var  <- (S_var  @ Wv_d) * B(x_{perm[p,d]})^2
    outputs: f_mean[n] = sum_{p,k} S_mean, f_var[n] = sum_{p,k} S_var / post_prec[p]

Device strategy (data-parallel over N across 8 cores, 3072 rows each):
  * state layout: (7 chains x 17 k -> 128 partitions incl. pad, n free),
    block-diagonal 128x128 bf16 chain matmuls (3 groups cover 20 chains).
    All PE traffic is bf16 (2x stream rate vs fp32r); PSUM accumulation fp32.
  * per-step Bernstein multipliers built in log space: one PE matmul contracts
    a baked selection/coefficient matrix A_{d,g} (128 x 128, bf16) against a
    resident hi/lo-split bf16 log-table UV (U_hi/V_hi/U_lo/V_lo, 128 x n)
    giving logM = k*log(x_c) + (16-k)*log(1-x_c) to ~16 mantissa bits; ACT
    computes M = exp(logM + log binom) -> bf16.
  * the PSUM->SBUF crossing is the bottleneck (DVE fp32 tensor_tensor runs at
    1 elem/cycle/lane): mean and var chain outputs land in separate
    single-buffered [128,1024] PSUM pools so each crossing is one FD-1024 op.
    A tunable fraction of tiles ("class C") reroutes the var crossing through
    ACT (copy PSUM->fp16 SBUF) + GPSIMD multiply, and squares M on DVE in
    16-bit 2x mode, which takes that work off the DVE critical path.
  * meanw0 / exp(varw0)*sc2 / sc2 / 1/post_prec folded host-side into the
    baked block-diagonal weights & reduction vectors.
  * emission is software-pipelined one tile ahead; final reduction matmuls are
    interleaved per chunk right after its d=31 tiles complete.
"""

import os
import numpy as np
import ml_dtypes
from math import comb

import concourse.bass as bass
import concourse.mybir as mybir
import concourse.tile as tile
from concourse import bacc
from concourse import bass_utils

ORDER = 16
K = 17
D = 32
P = 20
N = 24576
NCORES = 8
NLOC = N // NCORES        # 3072
CPG = 7                   # chain slots per group
G = 3                     # groups (7, 7, 6 + 1 pad)
R = CPG * K               # 119 active partitions
RP = 128                  # padded partition count
CHUNK = 1024
HALF = 512
F32 = mybir.dt.float32
BF16 = mybir.dt.bfloat16
F16 = mybir.dt.float16
EXP = mybir.ActivationFunctionType.Exp
MULT = mybir.AluOpType.mult


def _flags():
    c4 = int(os.environ.get("BB_C4", "0"))       # class-C tiles per 4
    evdt = os.environ.get("BB_EVDT", "f16")      # evac dtype f16|bf16
    sqdt = os.environ.get("BB_SQDT", "f16")      # class-C square dtype
    return c4, evdt, sqdt


def _bf16_split(x64):
    hi = x64.astype(ml_dtypes.bfloat16)
    lo = (x64 - hi.astype(np.float64)).astype(ml_dtypes.bfloat16)
    return hi, lo


def _host_tensors(Xnew, meanw0, meanw, varw0, varw, prior_sc, post_prec, perm):
    Xnew = np.asarray(Xnew, np.float32)
    meanw0 = np.asarray(meanw0, np.float64)   # (P, 1, K)
    meanw = np.asarray(meanw, np.float64)     # (D-1, P, K, K)
    varw0 = np.asarray(varw0, np.float64)     # (P, 1, K)
    varw = np.asarray(varw, np.float64)       # (D-1, P, K, K)
    prior_sc = np.asarray(prior_sc, np.float64)  # (K, 1)
    post_prec = np.asarray(post_prec, np.float64)  # (P,)
    perm = np.asarray(perm)                   # (P, D) int

    # --- per-core UV log tables (bf16 hi/lo split) --------------------
    x64 = np.clip(Xnew.astype(np.float64), 1e-30, None)
    u64 = np.log(x64)                                    # (N, D)
    v64 = np.log1p(-np.minimum(Xnew.astype(np.float64), 1.0 - 1e-15))
    uh, ul = _bf16_split(u64)
    vh, vl = _bf16_split(v64)
    uv_full = np.concatenate(
        [uh.T[None], vh.T[None], ul.T[None], vl.T[None]], axis=0
    )  # (4, D, N)
    uv_shards = []
    for i in range(NCORES):
        sl = uv_full[:, :, i * NLOC:(i + 1) * NLOC]      # (4, D, NLOC)
        uv_shards.append(
            np.ascontiguousarray(sl.reshape(4 * D, NLOC), ml_dtypes.bfloat16))

    # --- A selection/coefficient matrices (D*G, 128, RP), bf16 exact --
    ks = np.arange(K, dtype=np.float64)
    amat = np.zeros((D * G, 4 * D, RP), np.float64)
    for d in range(D):
        for g in range(G):
            A = amat[d * G + g]
            for c in range(CPG):
                p = g * CPG + c
                if p >= P:
                    continue
                col = perm[p, d]
                j = slice(K * c, K * c + K)
                A[col, j] = ks
                A[D + col, j] = ORDER - ks
                A[2 * D + col, j] = ks
                A[3 * D + col, j] = ORDER - ks
    amat = amat.astype(ml_dtypes.bfloat16)

    # --- block-diagonal chain weights (bf16) --------------------------
    sc2 = prior_sc[:, 0] ** 2                            # (K,)
    wmean = np.zeros(((D - 1) * G, RP, RP), np.float64)
    wvar = np.zeros(((D - 1) * G, RP, RP), np.float64)
    for d in range(1, D):
        for g in range(G):
            Wm = wmean[(d - 1) * G + g]
            Wv = wvar[(d - 1) * G + g]
            for c in range(CPG):
                p = g * CPG + c
                if p >= P:
                    continue
                blk = slice(K * c, K * c + K)
                m = meanw[d - 1, p]                      # (K, K) [k, j]
                v = np.exp(varw[d - 1, p]) * sc2[None, :]
                if d == 1:
                    m = meanw0[p, 0][:, None] * m
                    v = (np.exp(varw0[p, 0]) * sc2)[:, None] * v
                Wm[blk, blk] = m
                Wv[blk, blk] = v
    wmean = wmean.astype(ml_dtypes.bfloat16)
    wvar = wvar.astype(ml_dtypes.bfloat16)

    # --- reduction vectors (G, RP, 2): col0 mean ones, col1 var 1/pp --
    if np.all(post_prec > 0):
        qbar = float(np.exp(np.mean(np.log(1.0 / post_prec))))
    else:
        qbar = 1.0
    qbar_inv = (1.0 / post_prec) / qbar
    redw = np.zeros((G, RP, 2), np.float64)
    for g in range(G):
        for c in range(CPG):
            p = g * CPG + c
            if p >= P:
                continue
            blk = slice(K * c, K * c + K)
            redw[g, blk, 0] = 1.0
            redw[g, blk, 1] = qbar_inv[p]
    redw = redw.astype(ml_dtypes.bfloat16)

    # --- exp biases: log binom (per partition) ------------------------
    logb = np.log(np.array([comb(ORDER, k) for k in range(K)], np.float64))
    biasv = np.zeros((RP, 2), np.float64)
    biasv[:R, 0] = np.tile(logb, CPG)
    biasv[:R, 1] = 2.0 * np.tile(logb, CPG)
    biasv = biasv.astype(np.float32)

    shared = dict(amat=amat, wmean=wmean, wvar=wvar, redw=redw, biasv=biasv)
    return uv_shards, shared, qbar


def _build_module(nloc=NLOC):
    c4, evdt, sqdt = _flags()
    EV_DT = F16 if evdt == "f16" else BF16
    SQ_DT = F16 if sqdt == "f16" else BF16
    nchunk = max(1, nloc // CHUNK)
    chunk = min(CHUNK, nloc)
    rhalf = min(HALF, nloc)
    nh = chunk // rhalf                     # 512-halves per chunk

    nc = bacc.Bacc("TRN2", target_bir_lowering=False, debug=False)
    uv_d = nc.dram_tensor("uv", [4 * D, nloc], BF16, kind="ExternalInput").ap()
    amat_d = nc.dram_tensor("amat", [D * G, 4 * D, RP], BF16, kind="ExternalInput").ap()
    wm_d = nc.dram_tensor("wmean", [(D - 1) * G, RP, RP], BF16, kind="ExternalInput").ap()
    wv_d = nc.dram_tensor("wvar", [(D - 1) * G, RP, RP], BF16, kind="ExternalInput").ap()
    red_d = nc.dram_tensor("redw", [G, RP, 2], BF16, kind="ExternalInput").ap()
    bias_d = nc.dram_tensor("biasv", [RP, 2], F32, kind="ExternalInput").ap()
    out_d = nc.dram_tensor("out", [2, nloc], F32, kind="ExternalOutput").ap()

    tiles = [(d, g, ci) for d in range(D) for g in range(G) for ci in range(nchunk)]
    ntile = len(tiles)

    with tile.TileContext(nc) as tc:
        with (
            tc.tile_pool(name="persist", bufs=1) as persist,
            tc.tile_pool(name="wpool", bufs=4) as wpool,
            tc.tile_pool(name="mpool", bufs=4) as mpool,
            tc.tile_pool(name="psL", bufs=2, space="PSUM") as psL,
            tc.tile_pool(name="psM", bufs=1, space="PSUM") as psM,
            tc.tile_pool(name="psV", bufs=1, space="PSUM") as psV,
        ):
            uv = persist.tile([4 * D, nloc], BF16, tag="uv")
            for ci in range(nchunk):
                nc.sync.dma_start(
                    uv[:, ci * chunk:(ci + 1) * chunk],
                    uv_d[:, ci * chunk:(ci + 1) * chunk])
            bias = persist.tile([RP, 2], F32, tag="bias")
            nc.sync.dma_start(bias[:], bias_d)
            states = []
            for g in range(G):
                s = persist.tile([RP, nchunk, 2, chunk], BF16, tag=f"S{g}")
                states.append(s)
            redt = []
            for g in range(G):
                r = persist.tile([RP, 2], BF16, tag=f"RW{g}")
                nc.sync.dma_start(r[:], red_d[g])
                redt.append(r)
            outs = persist.tile([1, 2 * nloc], F32, tag="outs")

            loaded = {}

            def ensure_dg(t):
                if t >= ntile:
                    return
                d, g, _ = tiles[t]
                dg = d * G + g
                if dg in loaded:
                    return
                a_t = wpool.tile([4 * D, RP], BF16, tag="A")
                nc.sync.dma_start(a_t[:], amat_d[dg])
                entry = {"A": a_t}
                if d >= 1:
                    wm_t = wpool.tile([RP, RP], BF16, tag="WM")
                    nc.sync.dma_start(wm_t[:], wm_d[(d - 1) * G + g])
                    wv_t = wpool.tile([RP, RP], BF16, tag="WV")
                    nc.sync.dma_start(wv_t[:], wv_d[(d - 1) * G + g])
                    entry["WM"] = wm_t
                    entry["WV"] = wv_t
                loaded[dg] = entry

            pstore = {}

            def emit_gather(t):
                d, g, ci = tiles[t]
                a_t = loaded[d * G + g]["A"]
                ps = psL.tile([RP, chunk], F32, tag="L")
                pstore[t] = ps
                c0 = ci * chunk
                for h in range(nh):
                    nc.tensor.matmul(
                        ps[:, h * rhalf:(h + 1) * rhalf],
                        a_t[:],
                        uv[:, c0 + h * rhalf:c0 + (h + 1) * rhalf],
                        start=True, stop=True)

            def emit_reduction(ci):
                # f_mean/f_var partial sums for chunk ci (all groups at d=31).
                # pr lives in the psL (gather) pool: stealing a gather buffer
                # stalls only the PE/ACT side (slack), never the DVE var-mul
                # pipeline through psV.
                for h in range(nh):
                    off = ci * chunk + h * rhalf
                    o0 = h * rhalf
                    pr = psL.tile([1, 2, rhalf], F32, tag="L")
                    for g in range(G):
                        nc.tensor.matmul(
                            pr[:, 0, :], redt[g][:, 0:1],
                            states[g][:, ci, 0, o0:o0 + rhalf],
                            start=(g == 0), stop=(g == G - 1))
                    for g in range(G):
                        nc.tensor.matmul(
                            pr[:, 1, :], redt[g][:, 1:2],
                            states[g][:, ci, 1, o0:o0 + rhalf],
                            start=(g == 0), stop=(g == G - 1))
                    nc.scalar.copy(outs[0:1, off:off + rhalf], pr[:, 0, :])
                    nc.scalar.copy(
                        outs[0:1, nloc + off:nloc + off + rhalf], pr[:, 1, :])
                # ship this chunk's slice immediately (overlaps the tail)
                c0 = ci * chunk
                nc.sync.dma_start(
                    out_d[0:1, c0:c0 + chunk], outs[0:1, c0:c0 + chunk])
                nc.sync.dma_start(
                    out_d[1:2, c0:c0 + chunk],
                    outs[0:1, nloc + c0:nloc + c0 + chunk])

            def emit_compute(t):
                d, g, ci = tiles[t]
                ps = pstore.pop(t)
                S = states[g]
                if d == 0:
                    # initial states are the multipliers themselves; the
                    # square runs on the otherwise-idle DVE at startup
                    nc.scalar.activation(
                        S[:, ci, 0, :], ps[:], EXP,
                        bias=bias[:, 0:1], scale=1.0)
                    nc.vector.tensor_tensor(
                        S[:, ci, 1, :], S[:, ci, 0, :], S[:, ci, 0, :], MULT)
                    return
                ent = loaded[d * G + g]
                m_t = mpool.tile([RP, chunk], BF16, tag="M")
                nc.scalar.activation(
                    m_t[:], ps[:], EXP, bias=bias[:, 0:1], scale=1.0)
                is_c = (t % 4) < c4
                if is_c:
                    # class C: square on DVE (16-bit 2x), evac var PSUM via
                    # ACT to 16-bit SBUF, var multiply on GPSIMD
                    m2 = mpool.tile([RP, chunk], SQ_DT, tag="M2C")
                    nc.vector.tensor_tensor(m2[:], m_t[:], m_t[:], MULT)
                else:
                    # class A: square on GPSIMD to fp32, var multiply on DVE
                    m2 = mpool.tile([RP, chunk], F32, tag="M2A")
                    nc.gpsimd.tensor_tensor(m2[:], m_t[:], m_t[:], MULT)
                pcm = psM.tile([RP, chunk], F32, tag="M")
                pcv = psV.tile([RP, chunk], F32, tag="V")
                c0 = ci * chunk
                for h in range(nh):
                    hs = slice(h * rhalf, (h + 1) * rhalf)
                    nc.tensor.matmul(
                        pcm[:, hs], ent["WM"][:], S[:, ci, 0, hs],
                        start=True, stop=True)
                for h in range(nh):
                    hs = slice(h * rhalf, (h + 1) * rhalf)
                    nc.tensor.matmul(
                        pcv[:, hs], ent["WV"][:], S[:, ci, 1, hs],
                        start=True, stop=True)
                if is_c:
                    sbv = mpool.tile([RP, chunk], EV_DT, tag="SBV")
                    nc.scalar.copy(sbv[:], pcv[:])
                    nc.gpsimd.tensor_tensor(
                        S[:, ci, 1, :], sbv[:], m2[:], MULT)
                else:
                    nc.vector.tensor_tensor(
                        S[:, ci, 1, :], pcv[:], m2[:], MULT)
                nc.vector.tensor_tensor(
                    S[:, ci, 0, :], pcm[:], m_t[:], MULT)

            # software-pipelined emission: gather one tile ahead
            ensure_dg(0)
            emit_gather(0)
            done_last = 0
            for t in range(ntile):
                ensure_dg(t + 1)
                ensure_dg(t + nchunk + 1)    # prefetch next (d,g) weights
                if t + 1 < ntile:
                    emit_gather(t + 1)
                emit_compute(t)
                d, g, ci = tiles[t]
                if d == D - 1 and g == G - 1:
                    emit_reduction(ci)

    nc.compile()
    return nc


def kernel(Xnew, meanw0, meanw, varw0, varw, prior_sc, post_prec, perm):
    uv_shards, shared, qbar = _host_tensors(
        Xnew, meanw0, meanw, varw0, varw, prior_sc, post_prec, perm)
    nc = _build_module(NLOC)
    in_maps = [dict(uv=uv_shards[i], **shared) for i in range(NCORES)]
    res = bass_utils.run_bass_kernel_spmd(
        nc, in_maps, core_ids=list(range(NCORES)))
    outs = [res.results[i]["out"] for i in range(NCORES)]
    f_mean = np.concatenate([o[0] for o in outs]).reshape(N, 1).astype(np.float32)
    f_var = (np.concatenate([o[1] for o in outs]).reshape(N, 1)
             * np.float32(qbar)).astype(np.float32)
    return f_mean, f_var


# revision 11
# speedup vs baseline: 1.1704x; 1.1704x over previous
"""Trainium2 Bass kernel for nn_BezierButtress (Bernstein-basis permutation chains).

Math (per permutation chain p, over depth d = 0..31):
    S_mean <- (S_mean @ Wm_d) * B(x_{perm[p,d]})        (K=17 wide state)
    S_var  <- (S_var  @ Wv_d) * B(x_{perm[p,d]})^2
    outputs: f_mean[n] = sum_{p,k} S_mean, f_var[n] = sum_{p,k} S_var / post_prec[p]

Device strategy (data-parallel over N across 8 cores, 3072 rows each):
  * state layout: (7 chains x 17 k -> 128 partitions incl. pad, n free),
    block-diagonal 128x128 bf16 chain matmuls (3 groups cover 20 chains).
    All PE traffic is bf16 (2x stream rate vs fp32r); PSUM accumulation fp32.
  * per-step Bernstein multipliers built in log space: one PE matmul contracts
    a baked selection/coefficient matrix A_{d,g} (128 x 128, bf16) against a
    resident hi/lo-split bf16 log-table UV (U_hi/V_hi/U_lo/V_lo, 128 x n)
    giving logM = k*log(x_c) + (16-k)*log(1-x_c) to ~16 mantissa bits; ACT
    computes M = exp(logM + log binom) -> bf16.
  * the PSUM->SBUF crossing is the bottleneck (DVE fp32 tensor_tensor runs at
    1 elem/cycle/lane): mean and var chain outputs land in separate
    single-buffered [128,1024] PSUM pools so each crossing is one FD-1024 op.
    A tunable fraction of tiles ("class C") reroutes the var crossing through
    ACT (copy PSUM->fp16 SBUF) + GPSIMD multiply, and squares M on DVE in
    16-bit 2x mode, which takes that work off the DVE critical path.
  * meanw0 / exp(varw0)*sc2 / sc2 / 1/post_prec folded host-side into the
    baked block-diagonal weights & reduction vectors.
  * emission is software-pipelined one tile ahead; final reduction matmuls are
    interleaved per chunk right after its d=31 tiles complete.
"""

import os
import numpy as np
import ml_dtypes
from math import comb

import concourse.bass as bass
import concourse.mybir as mybir
import concourse.tile as tile
from concourse import bacc
from concourse import bass_utils

ORDER = 16
K = 17
D = 32
P = 20
N = 24576
NCORES = 8
NLOC = N // NCORES        # 3072
CPG = 7                   # chain slots per group
G = 3                     # groups (7, 7, 6 + 1 pad)
R = CPG * K               # 119 active partitions
RP = 128                  # padded partition count
CHUNK = 1024
HALF = 512
F32 = mybir.dt.float32
BF16 = mybir.dt.bfloat16
F16 = mybir.dt.float16
EXP = mybir.ActivationFunctionType.Exp
MULT = mybir.AluOpType.mult


def _flags():
    c4 = int(os.environ.get("BB_C4", "0"))       # class-C tiles per 4
    evdt = os.environ.get("BB_EVDT", "f16")      # evac dtype f16|bf16
    sqdt = os.environ.get("BB_SQDT", "f16")      # class-C square dtype
    return c4, evdt, sqdt


def _bf16_split(x64):
    hi = x64.astype(ml_dtypes.bfloat16)
    lo = (x64 - hi.astype(np.float64)).astype(ml_dtypes.bfloat16)
    return hi, lo


def _host_tensors(Xnew, meanw0, meanw, varw0, varw, prior_sc, post_prec, perm):
    Xnew = np.asarray(Xnew, np.float32)
    meanw0 = np.asarray(meanw0, np.float64)   # (P, 1, K)
    meanw = np.asarray(meanw, np.float64)     # (D-1, P, K, K)
    varw0 = np.asarray(varw0, np.float64)     # (P, 1, K)
    varw = np.asarray(varw, np.float64)       # (D-1, P, K, K)
    prior_sc = np.asarray(prior_sc, np.float64)  # (K, 1)
    post_prec = np.asarray(post_prec, np.float64)  # (P,)
    perm = np.asarray(perm)                   # (P, D) int

    # --- per-core UV log tables (bf16 hi/lo split) --------------------
    x64 = np.clip(Xnew.astype(np.float64), 1e-30, None)
    u64 = np.log(x64)                                    # (N, D)
    v64 = np.log1p(-np.minimum(Xnew.astype(np.float64), 1.0 - 1e-15))
    uh, ul = _bf16_split(u64)
    vh, vl = _bf16_split(v64)
    uv_full = np.concatenate(
        [uh.T[None], vh.T[None], ul.T[None], vl.T[None]], axis=0
    )  # (4, D, N)
    uv_shards = []
    for i in range(NCORES):
        sl = uv_full[:, :, i * NLOC:(i + 1) * NLOC]      # (4, D, NLOC)
        uv_shards.append(
            np.ascontiguousarray(sl.reshape(4 * D, NLOC), ml_dtypes.bfloat16))

    # --- A selection/coefficient matrices (D*G, 128, RP), bf16 exact --
    ks = np.arange(K, dtype=np.float64)
    amat = np.zeros((D * G, 4 * D, RP), np.float64)
    for d in range(D):
        for g in range(G):
            A = amat[d * G + g]
            for c in range(CPG):
                p = g * CPG + c
                if p >= P:
                    continue
                col = perm[p, d]
                j = slice(K * c, K * c + K)
                A[col, j] = ks
                A[D + col, j] = ORDER - ks
                A[2 * D + col, j] = ks
                A[3 * D + col, j] = ORDER - ks
    amat = amat.astype(ml_dtypes.bfloat16)

    # --- block-diagonal chain weights (bf16) --------------------------
    sc2 = prior_sc[:, 0] ** 2                            # (K,)
    wmean = np.zeros(((D - 1) * G, RP, RP), np.float64)
    wvar = np.zeros(((D - 1) * G, RP, RP), np.float64)
    for d in range(1, D):
        for g in range(G):
            Wm = wmean[(d - 1) * G + g]
            Wv = wvar[(d - 1) * G + g]
            for c in range(CPG):
                p = g * CPG + c
                if p >= P:
                    continue
                blk = slice(K * c, K * c + K)
                m = meanw[d - 1, p]                      # (K, K) [k, j]
                v = np.exp(varw[d - 1, p]) * sc2[None, :]
                if d == 1:
                    m = meanw0[p, 0][:, None] * m
                    v = (np.exp(varw0[p, 0]) * sc2)[:, None] * v
                Wm[blk, blk] = m
                Wv[blk, blk] = v
    wmean = wmean.astype(ml_dtypes.bfloat16)
    wvar = wvar.astype(ml_dtypes.bfloat16)

    # --- reduction vectors (G, RP, 2): col0 mean ones, col1 var 1/pp --
    if np.all(post_prec > 0):
        qbar = float(np.exp(np.mean(np.log(1.0 / post_prec))))
    else:
        qbar = 1.0
    qbar_inv = (1.0 / post_prec) / qbar
    redw = np.zeros((G, RP, 2), np.float64)
    for g in range(G):
        for c in range(CPG):
            p = g * CPG + c
            if p >= P:
                continue
            blk = slice(K * c, K * c + K)
            redw[g, blk, 0] = 1.0
            redw[g, blk, 1] = qbar_inv[p]
    redw = redw.astype(ml_dtypes.bfloat16)

    # --- exp biases: log binom (per partition) ------------------------
    logb = np.log(np.array([comb(ORDER, k) for k in range(K)], np.float64))
    biasv = np.zeros((RP, 2), np.float64)
    biasv[:R, 0] = np.tile(logb, CPG)
    biasv[:R, 1] = 2.0 * np.tile(logb, CPG)
    biasv = biasv.astype(np.float32)

    shared = dict(amat=amat, wmean=wmean, wvar=wvar, redw=redw, biasv=biasv)
    return uv_shards, shared, qbar


def _build_module(nloc=NLOC):
    c4, evdt, sqdt = _flags()
    EV_DT = F16 if evdt == "f16" else BF16
    SQ_DT = F16 if sqdt == "f16" else BF16
    nchunk = max(1, nloc // CHUNK)
    chunk = min(CHUNK, nloc)
    rhalf = min(HALF, nloc)
    nh = chunk // rhalf                     # 512-halves per chunk

    nc = bacc.Bacc("TRN2", target_bir_lowering=False, debug=False)
    uv_d = nc.dram_tensor("uv", [4 * D, nloc], BF16, kind="ExternalInput").ap()
    amat_d = nc.dram_tensor("amat", [D * G, 4 * D, RP], BF16, kind="ExternalInput").ap()
    wm_d = nc.dram_tensor("wmean", [(D - 1) * G, RP, RP], BF16, kind="ExternalInput").ap()
    wv_d = nc.dram_tensor("wvar", [(D - 1) * G, RP, RP], BF16, kind="ExternalInput").ap()
    red_d = nc.dram_tensor("redw", [G, RP, 2], BF16, kind="ExternalInput").ap()
    bias_d = nc.dram_tensor("biasv", [RP, 2], F32, kind="ExternalInput").ap()
    out_d = nc.dram_tensor("out", [2, nloc], F32, kind="ExternalOutput").ap()

    tiles = [(d, g, ci) for d in range(D) for g in range(G) for ci in range(nchunk)]
    ntile = len(tiles)

    with tile.TileContext(nc) as tc:
        with (
            tc.tile_pool(name="persist", bufs=1) as persist,
            tc.tile_pool(name="wpool", bufs=4) as wpool,
            tc.tile_pool(name="mpool", bufs=4) as mpool,
            tc.tile_pool(name="psL", bufs=2, space="PSUM") as psL,
            tc.tile_pool(name="psM", bufs=1, space="PSUM") as psM,
            tc.tile_pool(name="psV", bufs=1, space="PSUM") as psV,
        ):
            uv = persist.tile([4 * D, nloc], BF16, tag="uv")
            for ci in range(nchunk):
                nc.sync.dma_start(
                    uv[:, ci * chunk:(ci + 1) * chunk],
                    uv_d[:, ci * chunk:(ci + 1) * chunk])
            bias = persist.tile([RP, 2], F32, tag="bias")
            nc.sync.dma_start(bias[:], bias_d)
            states = []
            for g in range(G):
                s = persist.tile([RP, nchunk, 2, chunk], BF16, tag=f"S{g}")
                states.append(s)
            redt = []
            for g in range(G):
                r = persist.tile([RP, 2], BF16, tag=f"RW{g}")
                nc.sync.dma_start(r[:], red_d[g])
                redt.append(r)
            outs = persist.tile([1, 2 * nloc], F32, tag="outs")

            loaded = {}

            def ensure_dg(t):
                if t >= ntile:
                    return
                d, g, _ = tiles[t]
                dg = d * G + g
                if dg in loaded:
                    return
                a_t = wpool.tile([4 * D, RP], BF16, tag="A")
                nc.sync.dma_start(a_t[:], amat_d[dg])
                entry = {"A": a_t}
                if d >= 1:
                    wm_t = wpool.tile([RP, RP], BF16, tag="WM")
                    nc.sync.dma_start(wm_t[:], wm_d[(d - 1) * G + g])
                    wv_t = wpool.tile([RP, RP], BF16, tag="WV")
                    nc.sync.dma_start(wv_t[:], wv_d[(d - 1) * G + g])
                    entry["WM"] = wm_t
                    entry["WV"] = wv_t
                loaded[dg] = entry

            pstore = {}

            def emit_gather(t):
                d, g, ci = tiles[t]
                a_t = loaded[d * G + g]["A"]
                ps = psL.tile([RP, chunk], F32, tag="L")
                pstore[t] = ps
                c0 = ci * chunk
                for h in range(nh):
                    nc.tensor.matmul(
                        ps[:, h * rhalf:(h + 1) * rhalf],
                        a_t[:],
                        uv[:, c0 + h * rhalf:c0 + (h + 1) * rhalf],
                        start=True, stop=True)

            def emit_reduction(ci):
                # f_mean/f_var partial sums for chunk ci (all groups at d=31)
                for h in range(nh):
                    off = ci * chunk + h * rhalf
                    o0 = h * rhalf
                    pr = psV.tile([1, 2, rhalf], F32, tag="V")
                    for g in range(G):
                        nc.tensor.matmul(
                            pr[:, 0, :], redt[g][:, 0:1],
                            states[g][:, ci, 0, o0:o0 + rhalf],
                            start=(g == 0), stop=(g == G - 1))
                    for g in range(G):
                        nc.tensor.matmul(
                            pr[:, 1, :], redt[g][:, 1:2],
                            states[g][:, ci, 1, o0:o0 + rhalf],
                            start=(g == 0), stop=(g == G - 1))
                    nc.scalar.copy(outs[0:1, off:off + rhalf], pr[:, 0, :])
                    nc.scalar.copy(
                        outs[0:1, nloc + off:nloc + off + rhalf], pr[:, 1, :])

            def emit_compute(t):
                d, g, ci = tiles[t]
                ps = pstore.pop(t)
                S = states[g]
                if d == 0:
                    # initial states are the multipliers themselves
                    nc.scalar.activation(
                        S[:, ci, 0, :], ps[:], EXP,
                        bias=bias[:, 0:1], scale=1.0)
                    nc.gpsimd.tensor_tensor(
                        S[:, ci, 1, :], S[:, ci, 0, :], S[:, ci, 0, :], MULT)
                    return
                ent = loaded[d * G + g]
                m_t = mpool.tile([RP, chunk], BF16, tag="M")
                nc.scalar.activation(
                    m_t[:], ps[:], EXP, bias=bias[:, 0:1], scale=1.0)
                is_c = (t % 4) < c4
                if is_c:
                    # class C: square on DVE (16-bit 2x), evac var PSUM via
                    # ACT to 16-bit SBUF, var multiply on GPSIMD
                    m2 = mpool.tile([RP, chunk], SQ_DT, tag="M2C")
                    nc.vector.tensor_tensor(m2[:], m_t[:], m_t[:], MULT)
                else:
                    # class A: square on GPSIMD to fp32, var multiply on DVE
                    m2 = mpool.tile([RP, chunk], F32, tag="M2A")
                    nc.gpsimd.tensor_tensor(m2[:], m_t[:], m_t[:], MULT)
                pcm = psM.tile([RP, chunk], F32, tag="M")
                pcv = psV.tile([RP, chunk], F32, tag="V")
                c0 = ci * chunk
                for h in range(nh):
                    hs = slice(h * rhalf, (h + 1) * rhalf)
                    nc.tensor.matmul(
                        pcm[:, hs], ent["WM"][:], S[:, ci, 0, hs],
                        start=True, stop=True)
                for h in range(nh):
                    hs = slice(h * rhalf, (h + 1) * rhalf)
                    nc.tensor.matmul(
                        pcv[:, hs], ent["WV"][:], S[:, ci, 1, hs],
                        start=True, stop=True)
                if is_c:
                    sbv = mpool.tile([RP, chunk], EV_DT, tag="SBV")
                    nc.scalar.copy(sbv[:], pcv[:])
                    nc.gpsimd.tensor_tensor(
                        S[:, ci, 1, :], sbv[:], m2[:], MULT)
                else:
                    nc.vector.tensor_tensor(
                        S[:, ci, 1, :], pcv[:], m2[:], MULT)
                nc.vector.tensor_tensor(
                    S[:, ci, 0, :], pcm[:], m_t[:], MULT)

            # software-pipelined emission: gather one tile ahead
            ensure_dg(0)
            emit_gather(0)
            done_last = 0
            for t in range(ntile):
                ensure_dg(t + 1)
                ensure_dg(t + nchunk + 1)    # prefetch next (d,g) weights
                if t + 1 < ntile:
                    emit_gather(t + 1)
                emit_compute(t)
                d, g, ci = tiles[t]
                if d == D - 1 and g == G - 1:
                    emit_reduction(ci)

            nc.sync.dma_start(out_d.rearrange("a b -> (a b)")[None, :], outs[:])

    nc.compile()
    return nc


def kernel(Xnew, meanw0, meanw, varw0, varw, prior_sc, post_prec, perm):
    uv_shards, shared, qbar = _host_tensors(
        Xnew, meanw0, meanw, varw0, varw, prior_sc, post_prec, perm)
    nc = _build_module(NLOC)
    in_maps = [dict(uv=uv_shards[i], **shared) for i in range(NCORES)]
    res = bass_utils.run_bass_kernel_spmd(
        nc, in_maps, core_ids=list(range(NCORES)))
    outs = [res.results[i]["out"] for i in range(NCORES)]
    f_mean = np.concatenate([o[0] for o in outs]).reshape(N, 1).astype(np.float32)
    f_var = (np.concatenate([o[1] for o in outs]).reshape(N, 1)
             * np.float32(qbar)).astype(np.float32)
    return f_mean, f_var


# revision 14
# speedup vs baseline: 1.1775x; 1.0061x over previous
"""Trainium2 Bass kernel for nn_BezierButtress (Bernstein-basis permutation chains).

Math (per permutation chain p, over depth d = 0..31):
    S_mean <- (S_mean @ Wm_d) * B(x_{perm[p,d]})        (K=17 wide state)
    S_var  <- (S_var  @ Wv_d) * B(x_{perm[p,d]})^2
    outputs: f_mean[n] = sum_{p,k} S_mean, f_var[n] = sum_{p,k} S_var / post_prec[p]

Device strategy (data-parallel over N across 8 cores, 3072 rows each):
  * state layout: (7 chains x 17 k -> 128 partitions incl. pad, n free),
    block-diagonal 128x128 bf16 chain matmuls (3 groups cover 20 chains).
    All PE traffic is bf16 (2x stream rate vs fp32r); PSUM accumulation fp32.
  * per-step Bernstein multipliers built in log space: one PE matmul contracts
    a baked selection/coefficient matrix A_{d,g} (128 x 128, bf16) against a
    resident hi/lo-split bf16 log-table UV (U_hi/V_hi/U_lo/V_lo, 128 x n)
    giving logM = k*log(x_c) + (16-k)*log(1-x_c) to ~16 mantissa bits; ACT
    computes M = exp(logM + log binom) -> bf16.
  * the PSUM->SBUF crossing is the bottleneck (DVE fp32 tensor_tensor runs at
    1 elem/cycle/lane): mean and var chain outputs land in separate
    single-buffered [128,1024] PSUM pools so each crossing is one FD-1024 op.
    A tunable fraction of tiles ("class C") reroutes the var crossing through
    ACT (copy PSUM->fp16 SBUF) + GPSIMD multiply, and squares M on DVE in
    16-bit 2x mode, which takes that work off the DVE critical path.
  * meanw0 / exp(varw0)*sc2 / sc2 / 1/post_prec folded host-side into the
    baked block-diagonal weights & reduction vectors.
  * emission is software-pipelined one tile ahead; final reduction matmuls are
    interleaved per chunk right after its d=31 tiles complete.
"""

import os
import numpy as np
import ml_dtypes
from math import comb

import concourse.bass as bass
import concourse.mybir as mybir
import concourse.tile as tile
from concourse import bacc
from concourse import bass_utils

ORDER = 16
K = 17
D = 32
P = 20
N = 24576
NCORES = 8
NLOC = N // NCORES        # 3072
CPG = 7                   # chain slots per group
G = 3                     # groups (7, 7, 6 + 1 pad)
R = CPG * K               # 119 active partitions
RP = 128                  # padded partition count
CHUNK = 1024
HALF = 512
F32 = mybir.dt.float32
BF16 = mybir.dt.bfloat16
F16 = mybir.dt.float16
EXP = mybir.ActivationFunctionType.Exp
MULT = mybir.AluOpType.mult


def _flags():
    c4 = int(os.environ.get("BB_C4", "0"))       # class-C tiles per 4
    evdt = os.environ.get("BB_EVDT", "f16")      # evac dtype f16|bf16
    sqdt = os.environ.get("BB_SQDT", "f16")      # class-C square dtype
    return c4, evdt, sqdt


def _bf16_split(x64):
    hi = x64.astype(ml_dtypes.bfloat16)
    lo = (x64 - hi.astype(np.float64)).astype(ml_dtypes.bfloat16)
    return hi, lo


def _host_tensors(Xnew, meanw0, meanw, varw0, varw, prior_sc, post_prec, perm):
    Xnew = np.asarray(Xnew, np.float32)
    meanw0 = np.asarray(meanw0, np.float64)   # (P, 1, K)
    meanw = np.asarray(meanw, np.float64)     # (D-1, P, K, K)
    varw0 = np.asarray(varw0, np.float64)     # (P, 1, K)
    varw = np.asarray(varw, np.float64)       # (D-1, P, K, K)
    prior_sc = np.asarray(prior_sc, np.float64)  # (K, 1)
    post_prec = np.asarray(post_prec, np.float64)  # (P,)
    perm = np.asarray(perm)                   # (P, D) int

    # --- per-core UV log tables (bf16 hi/lo split) --------------------
    x64 = np.clip(Xnew.astype(np.float64), 1e-30, None)
    u64 = np.log(x64)                                    # (N, D)
    v64 = np.log1p(-np.minimum(Xnew.astype(np.float64), 1.0 - 1e-15))
    uh, ul = _bf16_split(u64)
    vh, vl = _bf16_split(v64)
    uv_full = np.concatenate(
        [uh.T[None], vh.T[None], ul.T[None], vl.T[None]], axis=0
    )  # (4, D, N)
    uv_shards = []
    for i in range(NCORES):
        sl = uv_full[:, :, i * NLOC:(i + 1) * NLOC]      # (4, D, NLOC)
        uv_shards.append(
            np.ascontiguousarray(sl.reshape(4 * D, NLOC), ml_dtypes.bfloat16))

    # --- A selection/coefficient matrices (D*G, 128, RP), bf16 exact --
    ks = np.arange(K, dtype=np.float64)
    amat = np.zeros((D * G, 4 * D, RP), np.float64)
    for d in range(D):
        for g in range(G):
            A = amat[d * G + g]
            for c in range(CPG):
                p = g * CPG + c
                if p >= P:
                    continue
                col = perm[p, d]
                j = slice(K * c, K * c + K)
                A[col, j] = ks
                A[D + col, j] = ORDER - ks
                A[2 * D + col, j] = ks
                A[3 * D + col, j] = ORDER - ks
    amat = amat.astype(ml_dtypes.bfloat16)

    # --- block-diagonal chain weights (bf16) --------------------------
    sc2 = prior_sc[:, 0] ** 2                            # (K,)
    wmean = np.zeros(((D - 1) * G, RP, RP), np.float64)
    wvar = np.zeros(((D - 1) * G, RP, RP), np.float64)
    for d in range(1, D):
        for g in range(G):
            Wm = wmean[(d - 1) * G + g]
            Wv = wvar[(d - 1) * G + g]
            for c in range(CPG):
                p = g * CPG + c
                if p >= P:
                    continue
                blk = slice(K * c, K * c + K)
                m = meanw[d - 1, p]                      # (K, K) [k, j]
                v = np.exp(varw[d - 1, p]) * sc2[None, :]
                if d == 1:
                    m = meanw0[p, 0][:, None] * m
                    v = (np.exp(varw0[p, 0]) * sc2)[:, None] * v
                Wm[blk, blk] = m
                Wv[blk, blk] = v
    wmean = wmean.astype(ml_dtypes.bfloat16)
    wvar = wvar.astype(ml_dtypes.bfloat16)

    # --- reduction vectors (G, RP, 2): col0 mean ones, col1 var 1/pp --
    if np.all(post_prec > 0):
        qbar = float(np.exp(np.mean(np.log(1.0 / post_prec))))
    else:
        qbar = 1.0
    qbar_inv = (1.0 / post_prec) / qbar
    redw = np.zeros((G, RP, 2), np.float64)
    for g in range(G):
        for c in range(CPG):
            p = g * CPG + c
            if p >= P:
                continue
            blk = slice(K * c, K * c + K)
            redw[g, blk, 0] = 1.0
            redw[g, blk, 1] = qbar_inv[p]
    redw = redw.astype(ml_dtypes.bfloat16)

    # --- exp biases: log binom (per partition) ------------------------
    logb = np.log(np.array([comb(ORDER, k) for k in range(K)], np.float64))
    biasv = np.zeros((RP, 2), np.float64)
    biasv[:R, 0] = np.tile(logb, CPG)
    biasv[:R, 1] = 2.0 * np.tile(logb, CPG)
    biasv = biasv.astype(np.float32)

    shared = dict(amat=amat, wmean=wmean, wvar=wvar, redw=redw, biasv=biasv)
    return uv_shards, shared, qbar


def _build_module(nloc=NLOC):
    c4, evdt, sqdt = _flags()
    EV_DT = F16 if evdt == "f16" else BF16
    SQ_DT = F16 if sqdt == "f16" else BF16
    nchunk = max(1, nloc // CHUNK)
    chunk = min(CHUNK, nloc)
    rhalf = min(HALF, nloc)
    nh = chunk // rhalf                     # 512-halves per chunk

    nc = bacc.Bacc("TRN2", target_bir_lowering=False, debug=False)
    uv_d = nc.dram_tensor("uv", [4 * D, nloc], BF16, kind="ExternalInput").ap()
    amat_d = nc.dram_tensor("amat", [D * G, 4 * D, RP], BF16, kind="ExternalInput").ap()
    wm_d = nc.dram_tensor("wmean", [(D - 1) * G, RP, RP], BF16, kind="ExternalInput").ap()
    wv_d = nc.dram_tensor("wvar", [(D - 1) * G, RP, RP], BF16, kind="ExternalInput").ap()
    red_d = nc.dram_tensor("redw", [G, RP, 2], BF16, kind="ExternalInput").ap()
    bias_d = nc.dram_tensor("biasv", [RP, 2], F32, kind="ExternalInput").ap()
    out_d = nc.dram_tensor("out", [2, nloc], F32, kind="ExternalOutput").ap()

    # wavefront order: tile (d, g, ci) runs at wave d + idx(g, ci).  Valid
    # (each column advances one depth per wave) and ramps the DVE up within
    # ~2 tiles instead of letting the nine d=0 tiles head-of-line-block the
    # ACT/GP FIFOs while the DVE idles (~20 us saved at startup).
    cols = [(g, ci) for g in range(G) for ci in range(nchunk)]
    cidx = {c: i for i, c in enumerate(cols)}
    tiles = [(d, g, ci) for d in range(D) for (g, ci) in cols]
    tiles.sort(key=lambda t: (t[0] + cidx[(t[1], t[2])], t[0]))
    ntile = len(tiles)

    with tile.TileContext(nc) as tc:
        with (
            tc.tile_pool(name="persist", bufs=1) as persist,
            tc.tile_pool(name="wpool", bufs=16) as wpool,
            tc.tile_pool(name="mpool", bufs=4) as mpool,
            tc.tile_pool(name="psL", bufs=2, space="PSUM") as psL,
            tc.tile_pool(name="psM", bufs=1, space="PSUM") as psM,
            tc.tile_pool(name="psV", bufs=1, space="PSUM") as psV,
        ):
            uv = persist.tile([4 * D, nloc], BF16, tag="uv")
            for ci in range(nchunk):
                nc.sync.dma_start(
                    uv[:, ci * chunk:(ci + 1) * chunk],
                    uv_d[:, ci * chunk:(ci + 1) * chunk])
            bias = persist.tile([RP, 2], F32, tag="bias")
            nc.sync.dma_start(bias[:], bias_d)
            states = []
            for g in range(G):
                s = persist.tile([RP, nchunk, 2, chunk], BF16, tag=f"S{g}")
                states.append(s)
            redt = []
            for g in range(G):
                r = persist.tile([RP, 2], BF16, tag=f"RW{g}")
                nc.sync.dma_start(r[:], red_d[g])
                redt.append(r)
            outs = persist.tile([1, 2 * nloc], F32, tag="outs")

            loaded = {}

            def ensure_dg(t):
                if t >= ntile:
                    return
                d, g, _ = tiles[t]
                dg = d * G + g
                if dg in loaded:
                    return
                a_t = wpool.tile([4 * D, RP], BF16, tag="A")
                nc.sync.dma_start(a_t[:], amat_d[dg])
                entry = {"A": a_t}
                if d >= 1:
                    wm_t = wpool.tile([RP, RP], BF16, tag="WM")
                    nc.sync.dma_start(wm_t[:], wm_d[(d - 1) * G + g])
                    wv_t = wpool.tile([RP, RP], BF16, tag="WV")
                    nc.sync.dma_start(wv_t[:], wv_d[(d - 1) * G + g])
                    entry["WM"] = wm_t
                    entry["WV"] = wv_t
                loaded[dg] = entry

            pstore = {}

            def emit_gather(t):
                d, g, ci = tiles[t]
                a_t = loaded[d * G + g]["A"]
                ps = psL.tile([RP, chunk], F32, tag="L")
                pstore[t] = ps
                c0 = ci * chunk
                for h in range(nh):
                    nc.tensor.matmul(
                        ps[:, h * rhalf:(h + 1) * rhalf],
                        a_t[:],
                        uv[:, c0 + h * rhalf:c0 + (h + 1) * rhalf],
                        start=True, stop=True)

            def emit_reduction(ci):
                # f_mean/f_var partial sums for chunk ci (all groups at d=31)
                for h in range(nh):
                    off = ci * chunk + h * rhalf
                    o0 = h * rhalf
                    pr = psV.tile([1, 2, rhalf], F32, tag="V")
                    for g in range(G):
                        nc.tensor.matmul(
                            pr[:, 0, :], redt[g][:, 0:1],
                            states[g][:, ci, 0, o0:o0 + rhalf],
                            start=(g == 0), stop=(g == G - 1))
                    for g in range(G):
                        nc.tensor.matmul(
                            pr[:, 1, :], redt[g][:, 1:2],
                            states[g][:, ci, 1, o0:o0 + rhalf],
                            start=(g == 0), stop=(g == G - 1))
                    nc.scalar.copy(outs[0:1, off:off + rhalf], pr[:, 0, :])
                    nc.scalar.copy(
                        outs[0:1, nloc + off:nloc + off + rhalf], pr[:, 1, :])

            def emit_compute(t):
                d, g, ci = tiles[t]
                ps = pstore.pop(t)
                S = states[g]
                if d == 0:
                    # initial states are the multipliers themselves
                    nc.scalar.activation(
                        S[:, ci, 0, :], ps[:], EXP,
                        bias=bias[:, 0:1], scale=1.0)
                    nc.gpsimd.tensor_tensor(
                        S[:, ci, 1, :], S[:, ci, 0, :], S[:, ci, 0, :], MULT)
                    return
                ent = loaded[d * G + g]
                m_t = mpool.tile([RP, chunk], BF16, tag="M")
                nc.scalar.activation(
                    m_t[:], ps[:], EXP, bias=bias[:, 0:1], scale=1.0)
                is_c = (t % 4) < c4
                if is_c:
                    # class C: square on DVE (16-bit 2x), evac var PSUM via
                    # ACT to 16-bit SBUF, var multiply on GPSIMD
                    m2 = mpool.tile([RP, chunk], SQ_DT, tag="M2C")
                    nc.vector.tensor_tensor(m2[:], m_t[:], m_t[:], MULT)
                else:
                    # class A: square on GPSIMD to fp32, var multiply on DVE
                    m2 = mpool.tile([RP, chunk], F32, tag="M2A")
                    nc.gpsimd.tensor_tensor(m2[:], m_t[:], m_t[:], MULT)
                pcm = psM.tile([RP, chunk], F32, tag="M")
                pcv = psV.tile([RP, chunk], F32, tag="V")
                c0 = ci * chunk
                for h in range(nh):
                    hs = slice(h * rhalf, (h + 1) * rhalf)
                    nc.tensor.matmul(
                        pcm[:, hs], ent["WM"][:], S[:, ci, 0, hs],
                        start=True, stop=True)
                for h in range(nh):
                    hs = slice(h * rhalf, (h + 1) * rhalf)
                    nc.tensor.matmul(
                        pcv[:, hs], ent["WV"][:], S[:, ci, 1, hs],
                        start=True, stop=True)
                if is_c:
                    sbv = mpool.tile([RP, chunk], EV_DT, tag="SBV")
                    nc.scalar.copy(sbv[:], pcv[:])
                    nc.gpsimd.tensor_tensor(
                        S[:, ci, 1, :], sbv[:], m2[:], MULT)
                else:
                    nc.vector.tensor_tensor(
                        S[:, ci, 1, :], pcv[:], m2[:], MULT)
                nc.vector.tensor_tensor(
                    S[:, ci, 0, :], pcm[:], m_t[:], MULT)

            # software-pipelined emission: gather one tile ahead
            ensure_dg(0)
            emit_gather(0)
            done_last = 0
            for t in range(ntile):
                # in wave order a (d,g) spans ~3 waves of 9 tiles; prefetch
                # far enough ahead that weights always arrive before use
                for k in range(1, 10):
                    ensure_dg(t + k)
                if t + 1 < ntile:
                    emit_gather(t + 1)
                emit_compute(t)
                d, g, ci = tiles[t]
                if d == D - 1 and g == G - 1:
                    emit_reduction(ci)

            nc.sync.dma_start(out_d.rearrange("a b -> (a b)")[None, :], outs[:])

    nc.compile()
    return nc


def kernel(Xnew, meanw0, meanw, varw0, varw, prior_sc, post_prec, perm):
    uv_shards, shared, qbar = _host_tensors(
        Xnew, meanw0, meanw, varw0, varw, prior_sc, post_prec, perm)
    nc = _build_module(NLOC)
    in_maps = [dict(uv=uv_shards[i], **shared) for i in range(NCORES)]
    res = bass_utils.run_bass_kernel_spmd(
        nc, in_maps, core_ids=list(range(NCORES)))
    outs = [res.results[i]["out"] for i in range(NCORES)]
    f_mean = np.concatenate([o[0] for o in outs]).reshape(N, 1).astype(np.float32)
    f_var = (np.concatenate([o[1] for o in outs]).reshape(N, 1)
             * np.float32(qbar)).astype(np.float32)
    return f_mean, f_var


# revision 17
# speedup vs baseline: 1.1786x; 1.0009x over previous
"""Trainium2 Bass kernel for nn_BezierButtress (Bernstein-basis permutation chains).

Math (per permutation chain p, over depth d = 0..31):
    S_mean <- (S_mean @ Wm_d) * B(x_{perm[p,d]})        (K=17 wide state)
    S_var  <- (S_var  @ Wv_d) * B(x_{perm[p,d]})^2
    outputs: f_mean[n] = sum_{p,k} S_mean, f_var[n] = sum_{p,k} S_var / post_prec[p]

Device strategy (data-parallel over N across 8 cores, 3072 rows each):
  * state layout: (7 chains x 17 k -> 128 partitions incl. pad, n free),
    block-diagonal 128x128 bf16 chain matmuls (3 groups cover 20 chains).
    All PE traffic is bf16 (2x stream rate vs fp32r); PSUM accumulation fp32.
  * per-step Bernstein multipliers built in log space: one PE matmul contracts
    a baked selection/coefficient matrix A_{d,g} (128 x 128, bf16) against a
    resident hi/lo-split bf16 log-table UV (U_hi/V_hi/U_lo/V_lo, 128 x n)
    giving logM = k*log(x_c) + (16-k)*log(1-x_c) to ~16 mantissa bits; ACT
    computes M = exp(logM + log binom) -> bf16.
  * the PSUM->SBUF crossing is the bottleneck (DVE fp32 tensor_tensor runs at
    1 elem/cycle/lane): mean and var chain outputs land in separate
    single-buffered [128,1024] PSUM pools so each crossing is one FD-1024 op.
    A tunable fraction of tiles ("class C") reroutes the var crossing through
    ACT (copy PSUM->fp16 SBUF) + GPSIMD multiply, and squares M on DVE in
    16-bit 2x mode, which takes that work off the DVE critical path.
  * meanw0 / exp(varw0)*sc2 / sc2 / 1/post_prec folded host-side into the
    baked block-diagonal weights & reduction vectors.
  * emission is software-pipelined one tile ahead; final reduction matmuls are
    interleaved per chunk right after its d=31 tiles complete.
"""

import os
import numpy as np
import ml_dtypes
from math import comb

import concourse.bass as bass
import concourse.mybir as mybir
import concourse.tile as tile
from concourse import bacc
from concourse import bass_utils

ORDER = 16
K = 17
D = 32
P = 20
N = 24576
NCORES = 8
NLOC = N // NCORES        # 3072
CPG = 7                   # chain slots per group
G = 3                     # groups (7, 7, 6 + 1 pad)
R = CPG * K               # 119 active partitions
RP = 128                  # padded partition count
CHUNK = 1024
HALF = 512
F32 = mybir.dt.float32
BF16 = mybir.dt.bfloat16
F16 = mybir.dt.float16
EXP = mybir.ActivationFunctionType.Exp
MULT = mybir.AluOpType.mult


def _flags():
    c4 = int(os.environ.get("BB_C4", "0"))       # class-C tiles per 4
    evdt = os.environ.get("BB_EVDT", "f16")      # evac dtype f16|bf16
    sqdt = os.environ.get("BB_SQDT", "f16")      # class-C square dtype
    return c4, evdt, sqdt


def _bf16_split(x64):
    hi = x64.astype(ml_dtypes.bfloat16)
    lo = (x64 - hi.astype(np.float64)).astype(ml_dtypes.bfloat16)
    return hi, lo


def _host_tensors(Xnew, meanw0, meanw, varw0, varw, prior_sc, post_prec, perm):
    Xnew = np.asarray(Xnew, np.float32)
    meanw0 = np.asarray(meanw0, np.float64)   # (P, 1, K)
    meanw = np.asarray(meanw, np.float64)     # (D-1, P, K, K)
    varw0 = np.asarray(varw0, np.float64)     # (P, 1, K)
    varw = np.asarray(varw, np.float64)       # (D-1, P, K, K)
    prior_sc = np.asarray(prior_sc, np.float64)  # (K, 1)
    post_prec = np.asarray(post_prec, np.float64)  # (P,)
    perm = np.asarray(perm)                   # (P, D) int

    # --- per-core UV log tables (bf16 hi/lo split) --------------------
    x64 = np.clip(Xnew.astype(np.float64), 1e-30, None)
    u64 = np.log(x64)                                    # (N, D)
    v64 = np.log1p(-np.minimum(Xnew.astype(np.float64), 1.0 - 1e-15))
    uh, ul = _bf16_split(u64)
    vh, vl = _bf16_split(v64)
    uv_full = np.concatenate(
        [uh.T[None], vh.T[None], ul.T[None], vl.T[None]], axis=0
    )  # (4, D, N)
    uv_shards = []
    for i in range(NCORES):
        sl = uv_full[:, :, i * NLOC:(i + 1) * NLOC]      # (4, D, NLOC)
        uv_shards.append(
            np.ascontiguousarray(sl.reshape(4 * D, NLOC), ml_dtypes.bfloat16))

    # --- A selection/coefficient matrices (D*G, 128, RP), bf16 exact --
    ks = np.arange(K, dtype=np.float64)
    amat = np.zeros((D * G, 4 * D, RP), np.float64)
    for d in range(D):
        for g in range(G):
            A = amat[d * G + g]
            for c in range(CPG):
                p = g * CPG + c
                if p >= P:
                    continue
                col = perm[p, d]
                j = slice(K * c, K * c + K)
                A[col, j] = ks
                A[D + col, j] = ORDER - ks
                A[2 * D + col, j] = ks
                A[3 * D + col, j] = ORDER - ks
    amat = amat.astype(ml_dtypes.bfloat16)

    # --- block-diagonal chain weights (bf16) --------------------------
    sc2 = prior_sc[:, 0] ** 2                            # (K,)
    wmean = np.zeros(((D - 1) * G, RP, RP), np.float64)
    wvar = np.zeros(((D - 1) * G, RP, RP), np.float64)
    for d in range(1, D):
        for g in range(G):
            Wm = wmean[(d - 1) * G + g]
            Wv = wvar[(d - 1) * G + g]
            for c in range(CPG):
                p = g * CPG + c
                if p >= P:
                    continue
                blk = slice(K * c, K * c + K)
                m = meanw[d - 1, p]                      # (K, K) [k, j]
                v = np.exp(varw[d - 1, p]) * sc2[None, :]
                if d == 1:
                    m = meanw0[p, 0][:, None] * m
                    v = (np.exp(varw0[p, 0]) * sc2)[:, None] * v
                Wm[blk, blk] = m
                Wv[blk, blk] = v
    wmean = wmean.astype(ml_dtypes.bfloat16)
    wvar = wvar.astype(ml_dtypes.bfloat16)

    # --- reduction vectors (G, RP, 2): col0 mean ones, col1 var 1/pp --
    if np.all(post_prec > 0):
        qbar = float(np.exp(np.mean(np.log(1.0 / post_prec))))
    else:
        qbar = 1.0
    qbar_inv = (1.0 / post_prec) / qbar
    redw = np.zeros((G, RP, 2), np.float64)
    for g in range(G):
        for c in range(CPG):
            p = g * CPG + c
            if p >= P:
                continue
            blk = slice(K * c, K * c + K)
            redw[g, blk, 0] = 1.0
            redw[g, blk, 1] = qbar_inv[p]
    redw = redw.astype(ml_dtypes.bfloat16)

    # --- exp biases: log binom (per partition) ------------------------
    logb = np.log(np.array([comb(ORDER, k) for k in range(K)], np.float64))
    biasv = np.zeros((RP, 2), np.float64)
    biasv[:R, 0] = np.tile(logb, CPG)
    biasv[:R, 1] = 2.0 * np.tile(logb, CPG)
    biasv = biasv.astype(np.float32)

    shared = dict(amat=amat, wmean=wmean, wvar=wvar, redw=redw, biasv=biasv)
    return uv_shards, shared, qbar


def _build_module(nloc=NLOC):
    c4, evdt, sqdt = _flags()
    EV_DT = F16 if evdt == "f16" else BF16
    SQ_DT = F16 if sqdt == "f16" else BF16
    nchunk = max(1, nloc // CHUNK)
    chunk = min(CHUNK, nloc)
    rhalf = min(HALF, nloc)
    nh = chunk // rhalf                     # 512-halves per chunk

    nc = bacc.Bacc("TRN2", target_bir_lowering=False, debug=False)
    uv_d = nc.dram_tensor("uv", [4 * D, nloc], BF16, kind="ExternalInput").ap()
    amat_d = nc.dram_tensor("amat", [D * G, 4 * D, RP], BF16, kind="ExternalInput").ap()
    wm_d = nc.dram_tensor("wmean", [(D - 1) * G, RP, RP], BF16, kind="ExternalInput").ap()
    wv_d = nc.dram_tensor("wvar", [(D - 1) * G, RP, RP], BF16, kind="ExternalInput").ap()
    red_d = nc.dram_tensor("redw", [G, RP, 2], BF16, kind="ExternalInput").ap()
    bias_d = nc.dram_tensor("biasv", [RP, 2], F32, kind="ExternalInput").ap()
    out_d = nc.dram_tensor("out", [2, nloc], F32, kind="ExternalOutput").ap()

    # wavefront order: tile (d, g, ci) runs at wave d + idx(g, ci).  Valid
    # (each column advances one depth per wave) and ramps the DVE up within
    # ~2 tiles instead of letting the nine d=0 tiles head-of-line-block the
    # ACT/GP FIFOs while the DVE idles (~20 us saved at startup).
    cols = [(g, ci) for g in range(G) for ci in range(nchunk)]
    cidx = {c: i for i, c in enumerate(cols)}
    tiles = [(d, g, ci) for d in range(D) for (g, ci) in cols]
    tiles.sort(key=lambda t: (t[0] + cidx[(t[1], t[2])], t[0]))
    ntile = len(tiles)

    with tile.TileContext(nc) as tc:
        with (
            tc.tile_pool(name="persist", bufs=1) as persist,
            tc.tile_pool(name="wpool", bufs=16) as wpool,
            tc.tile_pool(name="mpool", bufs=4) as mpool,
            tc.tile_pool(name="psL", bufs=2, space="PSUM") as psL,
            tc.tile_pool(name="psM", bufs=1, space="PSUM") as psM,
            tc.tile_pool(name="psV", bufs=1, space="PSUM") as psV,
        ):
            uv = persist.tile([4 * D, nloc], BF16, tag="uv")
            for ci in range(nchunk):
                nc.sync.dma_start(
                    uv[:, ci * chunk:(ci + 1) * chunk],
                    uv_d[:, ci * chunk:(ci + 1) * chunk])
            bias = persist.tile([RP, 2], F32, tag="bias")
            nc.sync.dma_start(bias[:], bias_d)
            states = []
            for g in range(G):
                s = persist.tile([RP, nchunk, 2, chunk], BF16, tag=f"S{g}")
                states.append(s)
            redt = []
            for g in range(G):
                r = persist.tile([RP, 2], BF16, tag=f"RW{g}")
                nc.sync.dma_start(r[:], red_d[g])
                redt.append(r)
            outs = persist.tile([1, 2 * nloc], F32, tag="outs")

            loaded = {}

            def ensure_dg(t):
                if t >= ntile:
                    return
                d, g, _ = tiles[t]
                dg = d * G + g
                if dg in loaded:
                    return
                a_t = wpool.tile([4 * D, RP], BF16, tag="A")
                nc.sync.dma_start(a_t[:], amat_d[dg])
                entry = {"A": a_t}
                if d >= 1:
                    wm_t = wpool.tile([RP, RP], BF16, tag="WM")
                    nc.sync.dma_start(wm_t[:], wm_d[(d - 1) * G + g])
                    wv_t = wpool.tile([RP, RP], BF16, tag="WV")
                    nc.sync.dma_start(wv_t[:], wv_d[(d - 1) * G + g])
                    entry["WM"] = wm_t
                    entry["WV"] = wv_t
                loaded[dg] = entry

            pstore = {}

            def emit_gather(t):
                d, g, ci = tiles[t]
                a_t = loaded[d * G + g]["A"]
                ps = psL.tile([RP, chunk], F32, tag="L")
                pstore[t] = ps
                c0 = ci * chunk
                for h in range(nh):
                    nc.tensor.matmul(
                        ps[:, h * rhalf:(h + 1) * rhalf],
                        a_t[:],
                        uv[:, c0 + h * rhalf:c0 + (h + 1) * rhalf],
                        start=True, stop=True)

            def emit_reduction(ci):
                # f_mean/f_var partial sums for chunk ci (all groups at d=31)
                for h in range(nh):
                    off = ci * chunk + h * rhalf
                    o0 = h * rhalf
                    pr = psV.tile([1, 2, rhalf], F32, tag="V")
                    for g in range(G):
                        nc.tensor.matmul(
                            pr[:, 0, :], redt[g][:, 0:1],
                            states[g][:, ci, 0, o0:o0 + rhalf],
                            start=(g == 0), stop=(g == G - 1))
                    for g in range(G):
                        nc.tensor.matmul(
                            pr[:, 1, :], redt[g][:, 1:2],
                            states[g][:, ci, 1, o0:o0 + rhalf],
                            start=(g == 0), stop=(g == G - 1))
                    nc.scalar.copy(outs[0:1, off:off + rhalf], pr[:, 0, :])
                    nc.scalar.copy(
                        outs[0:1, nloc + off:nloc + off + rhalf], pr[:, 1, :])
                # ship this chunk's slice now so the final DMA is off the tail
                c0 = ci * chunk
                nc.sync.dma_start(
                    out_d[0:1, c0:c0 + chunk], outs[0:1, c0:c0 + chunk])
                nc.sync.dma_start(
                    out_d[1:2, c0:c0 + chunk],
                    outs[0:1, nloc + c0:nloc + c0 + chunk])

            def emit_compute(t):
                d, g, ci = tiles[t]
                ps = pstore.pop(t)
                S = states[g]
                if d == 0:
                    # initial states are the multipliers themselves; square on
                    # the DVE, which is idle during the ramp waves, keeping the
                    # GP FIFO clear for the d>=1 squares
                    nc.scalar.activation(
                        S[:, ci, 0, :], ps[:], EXP,
                        bias=bias[:, 0:1], scale=1.0)
                    nc.vector.tensor_tensor(
                        S[:, ci, 1, :], S[:, ci, 0, :], S[:, ci, 0, :], MULT)
                    return
                ent = loaded[d * G + g]
                m_t = mpool.tile([RP, chunk], BF16, tag="M")
                nc.scalar.activation(
                    m_t[:], ps[:], EXP, bias=bias[:, 0:1], scale=1.0)
                is_c = (t % 4) < c4
                if is_c:
                    # class C: square on DVE (16-bit 2x), evac var PSUM via
                    # ACT to 16-bit SBUF, var multiply on GPSIMD
                    m2 = mpool.tile([RP, chunk], SQ_DT, tag="M2C")
                    nc.vector.tensor_tensor(m2[:], m_t[:], m_t[:], MULT)
                else:
                    # class A: square on GPSIMD to fp32, var multiply on DVE
                    m2 = mpool.tile([RP, chunk], F32, tag="M2A")
                    nc.gpsimd.tensor_tensor(m2[:], m_t[:], m_t[:], MULT)
                pcm = psM.tile([RP, chunk], F32, tag="M")
                pcv = psV.tile([RP, chunk], F32, tag="V")
                c0 = ci * chunk
                for h in range(nh):
                    hs = slice(h * rhalf, (h + 1) * rhalf)
                    nc.tensor.matmul(
                        pcm[:, hs], ent["WM"][:], S[:, ci, 0, hs],
                        start=True, stop=True)
                for h in range(nh):
                    hs = slice(h * rhalf, (h + 1) * rhalf)
                    nc.tensor.matmul(
                        pcv[:, hs], ent["WV"][:], S[:, ci, 1, hs],
                        start=True, stop=True)
                if is_c:
                    sbv = mpool.tile([RP, chunk], EV_DT, tag="SBV")
                    nc.scalar.copy(sbv[:], pcv[:])
                    nc.gpsimd.tensor_tensor(
                        S[:, ci, 1, :], sbv[:], m2[:], MULT)
                else:
                    nc.vector.tensor_tensor(
                        S[:, ci, 1, :], pcv[:], m2[:], MULT)
                nc.vector.tensor_tensor(
                    S[:, ci, 0, :], pcm[:], m_t[:], MULT)

            # software-pipelined emission: gather one tile ahead
            ensure_dg(0)
            emit_gather(0)
            done_last = 0
            for t in range(ntile):
                # in wave order a (d,g) spans ~3 waves of 9 tiles; prefetch
                # far enough ahead that weights always arrive before use
                for k in range(1, 10):
                    ensure_dg(t + k)
                if t + 1 < ntile:
                    emit_gather(t + 1)
                emit_compute(t)
                d, g, ci = tiles[t]
                if d == D - 1 and g == G - 1:
                    emit_reduction(ci)

    nc.compile()
    return nc


def kernel(Xnew, meanw0, meanw, varw0, varw, prior_sc, post_prec, perm):
    uv_shards, shared, qbar = _host_tensors(
        Xnew, meanw0, meanw, varw0, varw, prior_sc, post_prec, perm)
    nc = _build_module(NLOC)
    in_maps = [dict(uv=uv_shards[i], **shared) for i in range(NCORES)]
    res = bass_utils.run_bass_kernel_spmd(
        nc, in_maps, core_ids=list(range(NCORES)))
    outs = [res.results[i]["out"] for i in range(NCORES)]
    f_mean = np.concatenate([o[0] for o in outs]).reshape(N, 1).astype(np.float32)
    f_var = (np.concatenate([o[1] for o in outs]).reshape(N, 1)
             * np.float32(qbar)).astype(np.float32)
    return f_mean, f_var


# revision 20
# speedup vs baseline: 1.1850x; 1.0055x over previous
"""Trainium2 Bass kernel for nn_BezierButtress (Bernstein-basis permutation chains).

Math (per permutation chain p, over depth d = 0..31):
    S_mean <- (S_mean @ Wm_d) * B(x_{perm[p,d]})        (K=17 wide state)
    S_var  <- (S_var  @ Wv_d) * B(x_{perm[p,d]})^2
    outputs: f_mean[n] = sum_{p,k} S_mean, f_var[n] = sum_{p,k} S_var / post_prec[p]

Device strategy (data-parallel over N across 8 cores, 3072 rows each):
  * state layout: (7 chains x 17 k -> 128 partitions incl. pad, n free),
    block-diagonal 128x128 bf16 chain matmuls (3 groups cover 20 chains).
    All PE traffic is bf16 (2x stream rate vs fp32r); PSUM accumulation fp32.
  * per-step Bernstein multipliers built in log space: one PE matmul contracts
    a baked selection/coefficient matrix A_{d,g} (128 x 128, bf16) against a
    resident hi/lo-split bf16 log-table UV (U_hi/V_hi/U_lo/V_lo, 128 x n)
    giving logM = k*log(x_c) + (16-k)*log(1-x_c) to ~16 mantissa bits; ACT
    computes M = exp(logM + log binom) -> bf16.
  * the PSUM->SBUF crossing is the bottleneck (DVE fp32 tensor_tensor runs at
    1 elem/cycle/lane): mean and var chain outputs land in separate
    single-buffered [128,1024] PSUM pools so each crossing is one FD-1024 op.
    A tunable fraction of tiles ("class C") reroutes the var crossing through
    ACT (copy PSUM->fp16 SBUF) + GPSIMD multiply, and squares M on DVE in
    16-bit 2x mode, which takes that work off the DVE critical path.
  * meanw0 / exp(varw0)*sc2 / sc2 / 1/post_prec folded host-side into the
    baked block-diagonal weights & reduction vectors.
  * emission is software-pipelined one tile ahead; final reduction matmuls are
    interleaved per chunk right after its d=31 tiles complete.
"""

import os
import numpy as np
import ml_dtypes
from math import comb

import concourse.bass as bass
import concourse.mybir as mybir
import concourse.tile as tile
from concourse import bacc
from concourse import bass_utils

ORDER = 16
K = 17
D = 32
P = 20
N = 24576
NCORES = 8
NLOC = N // NCORES        # 3072
CPG = 7                   # chain slots per group
G = 3                     # groups (7, 7, 6 + 1 pad)
R = CPG * K               # 119 active partitions
RP = 128                  # padded partition count
CHUNK = 1024
HALF = 512
F32 = mybir.dt.float32
BF16 = mybir.dt.bfloat16
F16 = mybir.dt.float16
EXP = mybir.ActivationFunctionType.Exp
MULT = mybir.AluOpType.mult


def _flags():
    c4 = int(os.environ.get("BB_C4", "0"))       # class-C tiles per 4
    evdt = os.environ.get("BB_EVDT", "f16")      # evac dtype f16|bf16
    sqdt = os.environ.get("BB_SQDT", "f16")      # class-C square dtype
    return c4, evdt, sqdt


def _bf16_split(x64):
    hi = x64.astype(ml_dtypes.bfloat16)
    lo = (x64 - hi.astype(np.float64)).astype(ml_dtypes.bfloat16)
    return hi, lo


def _host_tensors(Xnew, meanw0, meanw, varw0, varw, prior_sc, post_prec, perm):
    Xnew = np.asarray(Xnew, np.float32)
    meanw0 = np.asarray(meanw0, np.float64)   # (P, 1, K)
    meanw = np.asarray(meanw, np.float64)     # (D-1, P, K, K)
    varw0 = np.asarray(varw0, np.float64)     # (P, 1, K)
    varw = np.asarray(varw, np.float64)       # (D-1, P, K, K)
    prior_sc = np.asarray(prior_sc, np.float64)  # (K, 1)
    post_prec = np.asarray(post_prec, np.float64)  # (P,)
    perm = np.asarray(perm)                   # (P, D) int

    # --- per-core UV log tables (bf16 hi/lo split) --------------------
    x64 = np.clip(Xnew.astype(np.float64), 1e-30, None)
    u64 = np.log(x64)                                    # (N, D)
    v64 = np.log1p(-np.minimum(Xnew.astype(np.float64), 1.0 - 1e-15))
    uh, ul = _bf16_split(u64)
    vh, vl = _bf16_split(v64)
    uv_full = np.concatenate(
        [uh.T[None], vh.T[None], ul.T[None], vl.T[None]], axis=0
    )  # (4, D, N)
    uv_shards = []
    for i in range(NCORES):
        sl = uv_full[:, :, i * NLOC:(i + 1) * NLOC]      # (4, D, NLOC)
        uv_shards.append(
            np.ascontiguousarray(sl.reshape(4 * D, NLOC), ml_dtypes.bfloat16))

    # --- A selection/coefficient matrices (D*G, 128, RP), bf16 exact --
    ks = np.arange(K, dtype=np.float64)
    amat = np.zeros((D * G, 4 * D, RP), np.float64)
    for d in range(D):
        for g in range(G):
            A = amat[d * G + g]
            for c in range(CPG):
                p = g * CPG + c
                if p >= P:
                    continue
                col = perm[p, d]
                j = slice(K * c, K * c + K)
                A[col, j] = ks
                A[D + col, j] = ORDER - ks
                A[2 * D + col, j] = ks
                A[3 * D + col, j] = ORDER - ks
    amat = amat.astype(ml_dtypes.bfloat16)

    # --- block-diagonal chain weights (bf16) --------------------------
    sc2 = prior_sc[:, 0] ** 2                            # (K,)
    wmean = np.zeros(((D - 1) * G, RP, RP), np.float64)
    wvar = np.zeros(((D - 1) * G, RP, RP), np.float64)
    for d in range(1, D):
        for g in range(G):
            Wm = wmean[(d - 1) * G + g]
            Wv = wvar[(d - 1) * G + g]
            for c in range(CPG):
                p = g * CPG + c
                if p >= P:
                    continue
                blk = slice(K * c, K * c + K)
                m = meanw[d - 1, p]                      # (K, K) [k, j]
                v = np.exp(varw[d - 1, p]) * sc2[None, :]
                if d == 1:
                    m = meanw0[p, 0][:, None] * m
                    v = (np.exp(varw0[p, 0]) * sc2)[:, None] * v
                Wm[blk, blk] = m
                Wv[blk, blk] = v
    wmean = wmean.astype(ml_dtypes.bfloat16)
    wvar = wvar.astype(ml_dtypes.bfloat16)

    # --- reduction vectors (G, RP, 2): col0 mean ones, col1 var 1/pp --
    if np.all(post_prec > 0):
        qbar = float(np.exp(np.mean(np.log(1.0 / post_prec))))
    else:
        qbar = 1.0
    qbar_inv = (1.0 / post_prec) / qbar
    redw = np.zeros((G, RP, 2), np.float64)
    for g in range(G):
        for c in range(CPG):
            p = g * CPG + c
            if p >= P:
                continue
            blk = slice(K * c, K * c + K)
            redw[g, blk, 0] = 1.0
            redw[g, blk, 1] = qbar_inv[p]
    redw = redw.astype(ml_dtypes.bfloat16)

    # --- exp biases: log binom (per partition) ------------------------
    logb = np.log(np.array([comb(ORDER, k) for k in range(K)], np.float64))
    biasv = np.zeros((RP, 2), np.float64)
    biasv[:R, 0] = np.tile(logb, CPG)
    biasv[:R, 1] = 2.0 * np.tile(logb, CPG)
    biasv = biasv.astype(np.float32)

    shared = dict(amat=amat, wmean=wmean, wvar=wvar, redw=redw, biasv=biasv)
    return uv_shards, shared, qbar


def _build_module(nloc=NLOC):
    c4, evdt, sqdt = _flags()
    EV_DT = F16 if evdt == "f16" else BF16
    SQ_DT = F16 if sqdt == "f16" else BF16
    nchunk = max(1, nloc // CHUNK)
    chunk = min(CHUNK, nloc)
    rhalf = min(HALF, nloc)
    nh = chunk // rhalf                     # 512-halves per chunk

    nc = bacc.Bacc("TRN2", target_bir_lowering=False, debug=False)
    uv_d = nc.dram_tensor("uv", [4 * D, nloc], BF16, kind="ExternalInput").ap()
    amat_d = nc.dram_tensor("amat", [D * G, 4 * D, RP], BF16, kind="ExternalInput").ap()
    wm_d = nc.dram_tensor("wmean", [(D - 1) * G, RP, RP], BF16, kind="ExternalInput").ap()
    wv_d = nc.dram_tensor("wvar", [(D - 1) * G, RP, RP], BF16, kind="ExternalInput").ap()
    red_d = nc.dram_tensor("redw", [G, RP, 2], BF16, kind="ExternalInput").ap()
    bias_d = nc.dram_tensor("biasv", [RP, 2], F32, kind="ExternalInput").ap()
    out_d = nc.dram_tensor("out", [2, nloc], F32, kind="ExternalOutput").ap()

    # wavefront order: tile (d, g, ci) runs at wave d + idx(g, ci).  Valid
    # (each column advances one depth per wave) and ramps the DVE up within
    # ~2 tiles instead of letting the nine d=0 tiles head-of-line-block the
    # ACT/GP FIFOs while the DVE idles (~20 us saved at startup).
    cols = [(g, ci) for g in range(G) for ci in range(nchunk)]
    cidx = {c: i for i, c in enumerate(cols)}
    tiles = [(d, g, ci) for d in range(D) for (g, ci) in cols]
    tiles.sort(key=lambda t: (t[0] + cidx[(t[1], t[2])], t[0]))
    ntile = len(tiles)

    with tile.TileContext(nc) as tc:
        with (
            tc.tile_pool(name="persist", bufs=1) as persist,
            tc.tile_pool(name="wpool", bufs=16) as wpool,
            tc.tile_pool(name="mpool", bufs=4) as mpool,
            tc.tile_pool(name="psL", bufs=2, space="PSUM") as psL,
            tc.tile_pool(name="psM", bufs=1, space="PSUM") as psM,
            tc.tile_pool(name="psV", bufs=1, space="PSUM") as psV,
        ):
            uv = persist.tile([4 * D, nloc], BF16, tag="uv")
            for ci in range(nchunk):
                nc.sync.dma_start(
                    uv[:, ci * chunk:(ci + 1) * chunk],
                    uv_d[:, ci * chunk:(ci + 1) * chunk])
            bias = persist.tile([RP, 2], F32, tag="bias")
            nc.sync.dma_start(bias[:], bias_d)
            # dummy exp at t~0 hoists the ~2.7us ACT table load off the
            # first real exp's critical path
            warm = persist.tile([1, 8], F32, tag="warm")
            nc.vector.memset(warm[:], 0.0)
            nc.scalar.activation(warm[:], warm[:], EXP)
            states = []
            for g in range(G):
                s = persist.tile([RP, nchunk, 2, chunk], BF16, tag=f"S{g}")
                states.append(s)
            redt = []
            for g in range(G):
                r = persist.tile([RP, 2], BF16, tag=f"RW{g}")
                nc.sync.dma_start(r[:], red_d[g])
                redt.append(r)
            outs = persist.tile([1, 2 * nloc], F32, tag="outs")

            loaded = {}

            def ensure_dg(t):
                if t >= ntile:
                    return
                d, g, _ = tiles[t]
                dg = d * G + g
                if dg in loaded:
                    return
                a_t = wpool.tile([4 * D, RP], BF16, tag="A")
                nc.sync.dma_start(a_t[:], amat_d[dg])
                entry = {"A": a_t}
                if d >= 1:
                    wm_t = wpool.tile([RP, RP], BF16, tag="WM")
                    nc.sync.dma_start(wm_t[:], wm_d[(d - 1) * G + g])
                    wv_t = wpool.tile([RP, RP], BF16, tag="WV")
                    nc.sync.dma_start(wv_t[:], wv_d[(d - 1) * G + g])
                    entry["WM"] = wm_t
                    entry["WV"] = wv_t
                loaded[dg] = entry

            pstore = {}

            def emit_gather(t):
                d, g, ci = tiles[t]
                a_t = loaded[d * G + g]["A"]
                ps = psL.tile([RP, chunk], F32, tag="L")
                pstore[t] = ps
                c0 = ci * chunk
                for h in range(nh):
                    nc.tensor.matmul(
                        ps[:, h * rhalf:(h + 1) * rhalf],
                        a_t[:],
                        uv[:, c0 + h * rhalf:c0 + (h + 1) * rhalf],
                        start=True, stop=True)

            def emit_reduction(ci):
                # f_mean/f_var partial sums for chunk ci (all groups at d=31)
                for h in range(nh):
                    off = ci * chunk + h * rhalf
                    o0 = h * rhalf
                    pr = psV.tile([1, 2, rhalf], F32, tag="V")
                    for g in range(G):
                        nc.tensor.matmul(
                            pr[:, 0, :], redt[g][:, 0:1],
                            states[g][:, ci, 0, o0:o0 + rhalf],
                            start=(g == 0), stop=(g == G - 1))
                    for g in range(G):
                        nc.tensor.matmul(
                            pr[:, 1, :], redt[g][:, 1:2],
                            states[g][:, ci, 1, o0:o0 + rhalf],
                            start=(g == 0), stop=(g == G - 1))
                    nc.scalar.copy(outs[0:1, off:off + rhalf], pr[:, 0, :])
                    nc.scalar.copy(
                        outs[0:1, nloc + off:nloc + off + rhalf], pr[:, 1, :])
                # ship this chunk's slice now so the final DMA is off the tail
                c0 = ci * chunk
                nc.sync.dma_start(
                    out_d[0:1, c0:c0 + chunk], outs[0:1, c0:c0 + chunk])
                nc.sync.dma_start(
                    out_d[1:2, c0:c0 + chunk],
                    outs[0:1, nloc + c0:nloc + c0 + chunk])

            def emit_compute(t):
                d, g, ci = tiles[t]
                ps = pstore.pop(t)
                S = states[g]
                if d == 0:
                    # initial states are the multipliers themselves; square on
                    # the DVE, which is idle during the ramp waves, keeping the
                    # GP FIFO clear for the d>=1 squares
                    nc.scalar.activation(
                        S[:, ci, 0, :], ps[:], EXP,
                        bias=bias[:, 0:1], scale=1.0)
                    nc.vector.tensor_tensor(
                        S[:, ci, 1, :], S[:, ci, 0, :], S[:, ci, 0, :], MULT)
                    return
                ent = loaded[d * G + g]
                m_t = mpool.tile([RP, chunk], BF16, tag="M")
                nc.scalar.activation(
                    m_t[:], ps[:], EXP, bias=bias[:, 0:1], scale=1.0)
                is_c = (t % 4) < c4
                if is_c:
                    # class C: square on DVE (16-bit 2x), evac var PSUM via
                    # ACT to 16-bit SBUF, var multiply on GPSIMD
                    m2 = mpool.tile([RP, chunk], SQ_DT, tag="M2C")
                    nc.vector.tensor_tensor(m2[:], m_t[:], m_t[:], MULT)
                else:
                    # class A: square on GPSIMD to fp32, var multiply on DVE
                    m2 = mpool.tile([RP, chunk], F32, tag="M2A")
                    nc.gpsimd.tensor_tensor(m2[:], m_t[:], m_t[:], MULT)
                pcm = psM.tile([RP, chunk], F32, tag="M")
                pcv = psV.tile([RP, chunk], F32, tag="V")
                c0 = ci * chunk
                for h in range(nh):
                    hs = slice(h * rhalf, (h + 1) * rhalf)
                    nc.tensor.matmul(
                        pcm[:, hs], ent["WM"][:], S[:, ci, 0, hs],
                        start=True, stop=True)
                for h in range(nh):
                    hs = slice(h * rhalf, (h + 1) * rhalf)
                    nc.tensor.matmul(
                        pcv[:, hs], ent["WV"][:], S[:, ci, 1, hs],
                        start=True, stop=True)
                if is_c:
                    sbv = mpool.tile([RP, chunk], EV_DT, tag="SBV")
                    nc.scalar.copy(sbv[:], pcv[:])
                    nc.gpsimd.tensor_tensor(
                        S[:, ci, 1, :], sbv[:], m2[:], MULT)
                else:
                    nc.vector.tensor_tensor(
                        S[:, ci, 1, :], pcv[:], m2[:], MULT)
                nc.vector.tensor_tensor(
                    S[:, ci, 0, :], pcm[:], m_t[:], MULT)

            # software-pipelined emission: gather one tile ahead
            ensure_dg(0)
            emit_gather(0)
            red_at = {}
            for t in range(ntile):
                # in wave order a (d,g) spans ~3 waves of 9 tiles; prefetch
                # far enough ahead that weights always arrive before use
                for k in range(1, 10):
                    ensure_dg(t + k)
                if t + 1 < ntile:
                    emit_gather(t + 1)
                emit_compute(t)
                d, g, ci = tiles[t]
                if d == D - 1 and g == G - 1:
                    # defer two tiles so the remaining columns' chain matmuls
                    # queue on the PE ahead of the reduction matmuls
                    red_at[t + 2] = ci
                if t in red_at:
                    emit_reduction(red_at.pop(t))
            for ci in sorted(red_at.values()):
                emit_reduction(ci)

    nc.compile()
    return nc


def kernel(Xnew, meanw0, meanw, varw0, varw, prior_sc, post_prec, perm):
    uv_shards, shared, qbar = _host_tensors(
        Xnew, meanw0, meanw, varw0, varw, prior_sc, post_prec, perm)
    nc = _build_module(NLOC)
    in_maps = [dict(uv=uv_shards[i], **shared) for i in range(NCORES)]
    res = bass_utils.run_bass_kernel_spmd(
        nc, in_maps, core_ids=list(range(NCORES)))
    outs = [res.results[i]["out"] for i in range(NCORES)]
    f_mean = np.concatenate([o[0] for o in outs]).reshape(N, 1).astype(np.float32)
    f_var = (np.concatenate([o[1] for o in outs]).reshape(N, 1)
             * np.float32(qbar)).astype(np.float32)
    return f_mean, f_var


# revision 23
# speedup vs baseline: 1.1864x; 1.0011x over previous
"""Trainium2 Bass kernel for nn_BezierButtress (Bernstein-basis permutation chains).

Math (per permutation chain p, over depth d = 0..31):
    S_mean <- (S_mean @ Wm_d) * B(x_{perm[p,d]})        (K=17 wide state)
    S_var  <- (S_var  @ Wv_d) * B(x_{perm[p,d]})^2
    outputs: f_mean[n] = sum_{p,k} S_mean, f_var[n] = sum_{p,k} S_var / post_prec[p]

Device strategy (data-parallel over N across 8 cores, 3072 rows each):
  * state layout: (7 chains x 17 k -> 128 partitions incl. pad, n free),
    block-diagonal 128x128 bf16 chain matmuls (3 groups cover 20 chains).
    All PE traffic is bf16 (2x stream rate vs fp32r); PSUM accumulation fp32.
  * per-step Bernstein multipliers built in log space: one PE matmul contracts
    a baked selection/coefficient matrix A_{d,g} (128 x 128, bf16) against a
    resident hi/lo-split bf16 log-table UV (U_hi/V_hi/U_lo/V_lo, 128 x n)
    giving logM = k*log(x_c) + (16-k)*log(1-x_c) to ~16 mantissa bits; ACT
    computes M = exp(logM + log binom) -> bf16.
  * the PSUM->SBUF crossing is the bottleneck (DVE fp32 tensor_tensor runs at
    1 elem/cycle/lane): mean and var chain outputs land in separate
    single-buffered [128,1024] PSUM pools so each crossing is one FD-1024 op.
    A tunable fraction of tiles ("class C") reroutes the var crossing through
    ACT (copy PSUM->fp16 SBUF) + GPSIMD multiply, and squares M on DVE in
    16-bit 2x mode, which takes that work off the DVE critical path.
  * meanw0 / exp(varw0)*sc2 / sc2 / 1/post_prec folded host-side into the
    baked block-diagonal weights & reduction vectors.
  * emission is software-pipelined one tile ahead; final reduction matmuls are
    interleaved per chunk right after its d=31 tiles complete.
"""

import os
import numpy as np
import ml_dtypes
from math import comb

import concourse.bass as bass
import concourse.mybir as mybir
import concourse.tile as tile
from concourse import bacc
from concourse import bass_utils

ORDER = 16
K = 17
D = 32
P = 20
N = 24576
NCORES = 8
NLOC = N // NCORES        # 3072
CPG = 7                   # chain slots per group
G = 3                     # groups (7, 7, 6 + 1 pad)
R = CPG * K               # 119 active partitions
RP = 128                  # padded partition count
CHUNK = 1024
HALF = 512
F32 = mybir.dt.float32
BF16 = mybir.dt.bfloat16
F16 = mybir.dt.float16
EXP = mybir.ActivationFunctionType.Exp
MULT = mybir.AluOpType.mult


def _flags():
    c4 = int(os.environ.get("BB_C4", "0"))       # class-C tiles per 4
    evdt = os.environ.get("BB_EVDT", "f16")      # evac dtype f16|bf16
    sqdt = os.environ.get("BB_SQDT", "f16")      # class-C square dtype
    return c4, evdt, sqdt


def _bf16_split(x64):
    hi = x64.astype(ml_dtypes.bfloat16)
    lo = (x64 - hi.astype(np.float64)).astype(ml_dtypes.bfloat16)
    return hi, lo


def _host_tensors(Xnew, meanw0, meanw, varw0, varw, prior_sc, post_prec, perm):
    Xnew = np.asarray(Xnew, np.float32)
    meanw0 = np.asarray(meanw0, np.float64)   # (P, 1, K)
    meanw = np.asarray(meanw, np.float64)     # (D-1, P, K, K)
    varw0 = np.asarray(varw0, np.float64)     # (P, 1, K)
    varw = np.asarray(varw, np.float64)       # (D-1, P, K, K)
    prior_sc = np.asarray(prior_sc, np.float64)  # (K, 1)
    post_prec = np.asarray(post_prec, np.float64)  # (P,)
    perm = np.asarray(perm)                   # (P, D) int

    # --- per-core UV log tables (bf16 hi/lo split) --------------------
    x64 = np.clip(Xnew.astype(np.float64), 1e-30, None)
    u64 = np.log(x64)                                    # (N, D)
    v64 = np.log1p(-np.minimum(Xnew.astype(np.float64), 1.0 - 1e-15))
    uh, ul = _bf16_split(u64)
    vh, vl = _bf16_split(v64)
    uv_full = np.concatenate(
        [uh.T[None], vh.T[None], ul.T[None], vl.T[None]], axis=0
    )  # (4, D, N)
    uv_shards = []
    for i in range(NCORES):
        sl = uv_full[:, :, i * NLOC:(i + 1) * NLOC]      # (4, D, NLOC)
        uv_shards.append(
            np.ascontiguousarray(sl.reshape(4 * D, NLOC), ml_dtypes.bfloat16))

    # --- A selection/coefficient matrices (D*G, 128, RP), bf16 exact --
    ks = np.arange(K, dtype=np.float64)
    amat = np.zeros((D * G, 4 * D, RP), np.float64)
    for d in range(D):
        for g in range(G):
            A = amat[d * G + g]
            for c in range(CPG):
                p = g * CPG + c
                if p >= P:
                    continue
                col = perm[p, d]
                j = slice(K * c, K * c + K)
                A[col, j] = ks
                A[D + col, j] = ORDER - ks
                A[2 * D + col, j] = ks
                A[3 * D + col, j] = ORDER - ks
    amat = amat.astype(ml_dtypes.bfloat16)

    # --- block-diagonal chain weights (bf16) --------------------------
    sc2 = prior_sc[:, 0] ** 2                            # (K,)
    wmean = np.zeros(((D - 1) * G, RP, RP), np.float64)
    wvar = np.zeros(((D - 1) * G, RP, RP), np.float64)
    for d in range(1, D):
        for g in range(G):
            Wm = wmean[(d - 1) * G + g]
            Wv = wvar[(d - 1) * G + g]
            for c in range(CPG):
                p = g * CPG + c
                if p >= P:
                    continue
                blk = slice(K * c, K * c + K)
                m = meanw[d - 1, p]                      # (K, K) [k, j]
                v = np.exp(varw[d - 1, p]) * sc2[None, :]
                if d == 1:
                    m = meanw0[p, 0][:, None] * m
                    v = (np.exp(varw0[p, 0]) * sc2)[:, None] * v
                Wm[blk, blk] = m
                Wv[blk, blk] = v
    wmean = wmean.astype(ml_dtypes.bfloat16)
    wvar = wvar.astype(ml_dtypes.bfloat16)

    # --- reduction vectors (G, RP, 2): col0 mean ones, col1 var 1/pp --
    if np.all(post_prec > 0):
        qbar = float(np.exp(np.mean(np.log(1.0 / post_prec))))
    else:
        qbar = 1.0
    qbar_inv = (1.0 / post_prec) / qbar
    redw = np.zeros((G, RP, 2), np.float64)
    for g in range(G):
        for c in range(CPG):
            p = g * CPG + c
            if p >= P:
                continue
            blk = slice(K * c, K * c + K)
            redw[g, blk, 0] = 1.0
            redw[g, blk, 1] = qbar_inv[p]
    redw = redw.astype(ml_dtypes.bfloat16)

    # --- exp biases: log binom (per partition) ------------------------
    logb = np.log(np.array([comb(ORDER, k) for k in range(K)], np.float64))
    biasv = np.zeros((RP, 2), np.float64)
    biasv[:R, 0] = np.tile(logb, CPG)
    biasv[:R, 1] = 2.0 * np.tile(logb, CPG)
    biasv = biasv.astype(np.float32)

    shared = dict(amat=amat, wmean=wmean, wvar=wvar, redw=redw, biasv=biasv)
    return uv_shards, shared, qbar


def _build_module(nloc=NLOC):
    c4, evdt, sqdt = _flags()
    EV_DT = F16 if evdt == "f16" else BF16
    SQ_DT = F16 if sqdt == "f16" else BF16
    nchunk = max(1, nloc // CHUNK)
    chunk = min(CHUNK, nloc)
    rhalf = min(HALF, nloc)
    nh = chunk // rhalf                     # 512-halves per chunk

    nc = bacc.Bacc("TRN2", target_bir_lowering=False, debug=False)
    uv_d = nc.dram_tensor("uv", [4 * D, nloc], BF16, kind="ExternalInput").ap()
    amat_d = nc.dram_tensor("amat", [D * G, 4 * D, RP], BF16, kind="ExternalInput").ap()
    wm_d = nc.dram_tensor("wmean", [(D - 1) * G, RP, RP], BF16, kind="ExternalInput").ap()
    wv_d = nc.dram_tensor("wvar", [(D - 1) * G, RP, RP], BF16, kind="ExternalInput").ap()
    red_d = nc.dram_tensor("redw", [G, RP, 2], BF16, kind="ExternalInput").ap()
    bias_d = nc.dram_tensor("biasv", [RP, 2], F32, kind="ExternalInput").ap()
    out_d = nc.dram_tensor("out", [2, nloc], F32, kind="ExternalOutput").ap()

    # wavefront order: tile (d, g, ci) runs at wave d + idx(g, ci).  Valid
    # (each column advances one depth per wave) and ramps the DVE up within
    # ~2 tiles instead of letting the nine d=0 tiles head-of-line-block the
    # ACT/GP FIFOs while the DVE idles (~20 us saved at startup).
    cols = [(g, ci) for g in range(G) for ci in range(nchunk)]
    cidx = {c: i for i, c in enumerate(cols)}
    tiles = [(d, g, ci) for d in range(D) for (g, ci) in cols]
    tiles.sort(key=lambda t: (t[0] + cidx[(t[1], t[2])], t[0]))
    ntile = len(tiles)

    with tile.TileContext(nc) as tc:
        with (
            tc.tile_pool(name="persist", bufs=1) as persist,
            tc.tile_pool(name="wpool", bufs=16) as wpool,
            tc.tile_pool(name="mpool", bufs=4) as mpool,
            tc.tile_pool(name="psL", bufs=2, space="PSUM") as psL,
            tc.tile_pool(name="psM", bufs=1, space="PSUM") as psM,
            tc.tile_pool(name="psV", bufs=1, space="PSUM") as psV,
        ):
            # critical-path startup DMAs first: uv chunk 0 + bias, then the
            # first A matrix (ensure_dg(0) below); remaining uv chunks and
            # redw queue after the first gather is unblocked
            uv = persist.tile([4 * D, nloc], BF16, tag="uv")
            nc.sync.dma_start(uv[:, 0:chunk], uv_d[:, 0:chunk])
            bias = persist.tile([RP, 2], F32, tag="bias")
            nc.sync.dma_start(bias[:], bias_d)
            # dummy exp at t~0 hoists the ~2.7us ACT table load off the
            # first real exp's critical path
            warm = persist.tile([1, 8], F32, tag="warm")
            nc.vector.memset(warm[:], 0.0)
            nc.scalar.activation(warm[:], warm[:], EXP)
            states = []
            for g in range(G):
                s = persist.tile([RP, nchunk, 2, chunk], BF16, tag=f"S{g}")
                states.append(s)
            redt = []
            for g in range(G):
                r = persist.tile([RP, 2], BF16, tag=f"RW{g}")
                redt.append(r)
            outs = persist.tile([1, 2 * nloc], F32, tag="outs")

            loaded = {}

            def ensure_dg(t):
                if t >= ntile:
                    return
                d, g, _ = tiles[t]
                dg = d * G + g
                if dg in loaded:
                    return
                a_t = wpool.tile([4 * D, RP], BF16, tag="A")
                nc.sync.dma_start(a_t[:], amat_d[dg])
                entry = {"A": a_t}
                if d >= 1:
                    wm_t = wpool.tile([RP, RP], BF16, tag="WM")
                    nc.sync.dma_start(wm_t[:], wm_d[(d - 1) * G + g])
                    wv_t = wpool.tile([RP, RP], BF16, tag="WV")
                    nc.sync.dma_start(wv_t[:], wv_d[(d - 1) * G + g])
                    entry["WM"] = wm_t
                    entry["WV"] = wv_t
                loaded[dg] = entry

            pstore = {}

            def emit_gather(t):
                d, g, ci = tiles[t]
                a_t = loaded[d * G + g]["A"]
                ps = psL.tile([RP, chunk], F32, tag="L")
                pstore[t] = ps
                c0 = ci * chunk
                for h in range(nh):
                    nc.tensor.matmul(
                        ps[:, h * rhalf:(h + 1) * rhalf],
                        a_t[:],
                        uv[:, c0 + h * rhalf:c0 + (h + 1) * rhalf],
                        start=True, stop=True)

            def emit_reduction(ci):
                # f_mean/f_var partial sums for chunk ci (all groups at d=31)
                for h in range(nh):
                    off = ci * chunk + h * rhalf
                    o0 = h * rhalf
                    pr = psV.tile([1, 2, rhalf], F32, tag="V")
                    for g in range(G):
                        nc.tensor.matmul(
                            pr[:, 0, :], redt[g][:, 0:1],
                            states[g][:, ci, 0, o0:o0 + rhalf],
                            start=(g == 0), stop=(g == G - 1))
                    for g in range(G):
                        nc.tensor.matmul(
                            pr[:, 1, :], redt[g][:, 1:2],
                            states[g][:, ci, 1, o0:o0 + rhalf],
                            start=(g == 0), stop=(g == G - 1))
                    nc.scalar.copy(outs[0:1, off:off + rhalf], pr[:, 0, :])
                    nc.scalar.copy(
                        outs[0:1, nloc + off:nloc + off + rhalf], pr[:, 1, :])
                # ship this chunk's slice now so the final DMA is off the tail
                c0 = ci * chunk
                nc.sync.dma_start(
                    out_d[0:1, c0:c0 + chunk], outs[0:1, c0:c0 + chunk])
                nc.sync.dma_start(
                    out_d[1:2, c0:c0 + chunk],
                    outs[0:1, nloc + c0:nloc + c0 + chunk])

            def emit_compute(t):
                d, g, ci = tiles[t]
                ps = pstore.pop(t)
                S = states[g]
                if d == 0:
                    # initial states are the multipliers themselves; square on
                    # the DVE, which is idle during the ramp waves, keeping the
                    # GP FIFO clear for the d>=1 squares
                    nc.scalar.activation(
                        S[:, ci, 0, :], ps[:], EXP,
                        bias=bias[:, 0:1], scale=1.0)
                    nc.vector.tensor_tensor(
                        S[:, ci, 1, :], S[:, ci, 0, :], S[:, ci, 0, :], MULT)
                    return
                ent = loaded[d * G + g]
                m_t = mpool.tile([RP, chunk], BF16, tag="M")
                nc.scalar.activation(
                    m_t[:], ps[:], EXP, bias=bias[:, 0:1], scale=1.0)
                is_c = (t % 4) < c4
                if is_c:
                    # class C: square on DVE (16-bit 2x), evac var PSUM via
                    # ACT to 16-bit SBUF, var multiply on GPSIMD
                    m2 = mpool.tile([RP, chunk], SQ_DT, tag="M2C")
                    nc.vector.tensor_tensor(m2[:], m_t[:], m_t[:], MULT)
                else:
                    # class A: square on GPSIMD to fp32, var multiply on DVE
                    m2 = mpool.tile([RP, chunk], F32, tag="M2A")
                    nc.gpsimd.tensor_tensor(m2[:], m_t[:], m_t[:], MULT)
                pcm = psM.tile([RP, chunk], F32, tag="M")
                pcv = psV.tile([RP, chunk], F32, tag="V")
                c0 = ci * chunk
                for h in range(nh):
                    hs = slice(h * rhalf, (h + 1) * rhalf)
                    nc.tensor.matmul(
                        pcm[:, hs], ent["WM"][:], S[:, ci, 0, hs],
                        start=True, stop=True)
                for h in range(nh):
                    hs = slice(h * rhalf, (h + 1) * rhalf)
                    nc.tensor.matmul(
                        pcv[:, hs], ent["WV"][:], S[:, ci, 1, hs],
                        start=True, stop=True)
                if is_c:
                    sbv = mpool.tile([RP, chunk], EV_DT, tag="SBV")
                    nc.scalar.copy(sbv[:], pcv[:])
                    nc.gpsimd.tensor_tensor(
                        S[:, ci, 1, :], sbv[:], m2[:], MULT)
                else:
                    nc.vector.tensor_tensor(
                        S[:, ci, 1, :], pcv[:], m2[:], MULT)
                nc.vector.tensor_tensor(
                    S[:, ci, 0, :], pcm[:], m_t[:], MULT)

            # software-pipelined emission: gather one tile ahead
            ensure_dg(0)
            emit_gather(0)
            for ci in range(1, nchunk):
                nc.sync.dma_start(
                    uv[:, ci * chunk:(ci + 1) * chunk],
                    uv_d[:, ci * chunk:(ci + 1) * chunk])
            for g in range(G):
                nc.sync.dma_start(redt[g][:], red_d[g])
            red_at = {}
            for t in range(ntile):
                # in wave order a (d,g) spans ~3 waves of 9 tiles; prefetch
                # far enough ahead that weights always arrive before use
                for k in range(1, 10):
                    ensure_dg(t + k)
                if t + 1 < ntile:
                    emit_gather(t + 1)
                emit_compute(t)
                d, g, ci = tiles[t]
                if d == D - 1 and g == G - 1:
                    # defer two tiles so the remaining columns' chain matmuls
                    # queue on the PE ahead of the reduction matmuls
                    red_at[t + 2] = ci
                if t in red_at:
                    emit_reduction(red_at.pop(t))
            for ci in sorted(red_at.values()):
                emit_reduction(ci)

    nc.compile()
    return nc


def kernel(Xnew, meanw0, meanw, varw0, varw, prior_sc, post_prec, perm):
    uv_shards, shared, qbar = _host_tensors(
        Xnew, meanw0, meanw, varw0, varw, prior_sc, post_prec, perm)
    nc = _build_module(NLOC)
    in_maps = [dict(uv=uv_shards[i], **shared) for i in range(NCORES)]
    res = bass_utils.run_bass_kernel_spmd(
        nc, in_maps, core_ids=list(range(NCORES)))
    outs = [res.results[i]["out"] for i in range(NCORES)]
    f_mean = np.concatenate([o[0] for o in outs]).reshape(N, 1).astype(np.float32)
    f_var = (np.concatenate([o[1] for o in outs]).reshape(N, 1)
             * np.float32(qbar)).astype(np.float32)
    return f_mean, f_var
